# revision 1
# baseline (speedup 1.0000x reference)
"""Trainium2 Bass kernel for nn_ARPredVideoVanilla (8-core data-parallel).

Strategy: pure data parallelism over batch (B=8 -> 1 batch element per core,
no collectives).  Activations live feature-major in SBUF: (128 partitions =
feature chunk, KO feature tiles, tokens free).  Weights are pre-folded on the
host (LN scale/bias folded into the following matmul, attention scale folded
into W_q, K-bias dropped (softmax row-shift invariant), V-bias folded into the
output-projection bias) and shipped as bf16.  The stochastic block mask is
evaluated on the host and compiled into the graph: blocked (query,key) frame
blocks are simply never computed.  Softmax runs without max-subtraction
(scores are O(1) by construction); exp runs on ScalarE with accum_out row sums.
"""

import sys

sys.path.insert(0, "/opt/trn_rl_repo")

import numpy as np
import ml_dtypes

BF16 = ml_dtypes.bfloat16

# ---- model dims (hardcoded from the problem spec) ----
B, T, V = 8, 4, 3
C = V * T                      # 12
H, W, PH, PW = 128, 256, 16, 16
HP, WP = H // PH, W // PW      # 8, 16
P = HP * WP                    # 128 patches/frame
S = T * P                      # 512
D, NH, HD, L = 768, 12, 64, 8
DD, NHD, HDD, LD = 512, 8, 64, 4
MLP, MLPD = 3072, 2048
OUT = PH * PW * V              # 768
MASK_RATIO = 0.8
EPS = 1e-5
PP = 128  # partitions


# ------------------------------------------------------------------
# host-side preparation: fold biases/scales, transpose, cast to bf16
# ------------------------------------------------------------------

def _prep(inputs):
    f32 = np.float32
    g = {k: np.asarray(v, dtype=f32) for k, v in inputs.items()}

    w = {}

    def bf(a):
        return np.ascontiguousarray(a.astype(BF16))

    def pcol(bias):  # (M,) -> (128, M//128) per-partition layout, tile-major
        M = bias.shape[0]
        return np.ascontiguousarray(bias.reshape(M // PP, PP).T.astype(f32))

    # patch data, per core: x[b] (T,C,H,W) -> xfT (C*PH*PW, T*P)
    x = g["x"]  # (B,T,C,H,W)
    xf = x.reshape(B, T, C, HP, PH, WP, PW).transpose(0, 1, 3, 5, 2, 4, 6)
    xf = xf.reshape(B, T * P, C * PH * PW)          # (B, 512, 3072)
    xfT = np.ascontiguousarray(np.swapaxes(xf, 1, 2))  # (B, 3072, 512)
    xfT_bf = [bf(xfT[b]) for b in range(B)]

    # conv: wk (3072, 768); pos_eff (768, 512) f32 with conv_b folded in
    wk = g["conv_w"].reshape(D, C * PH * PW).T      # (3072, 768)
    w["wk"] = bf(wk)
    pos = g["pos_emb"][0].T + g["conv_b"][:, None]  # (768, 512)
    w["pos"] = np.ascontiguousarray(pos.astype(f32))

    scale = HD ** -0.5
    for l in range(L):
        s1, b1 = g["enc_ln1_s"][l], g["enc_ln1_b"][l]
        Wqkv = g["enc_qkv_w"][l]                    # (768, 2304)
        Wq_eff = (s1[:, None] * Wqkv).copy()
        Wq_eff[:, :D] *= scale
        w[f"eqk{l}"] = bf(Wq_eff[:, :2 * D])
        w[f"evw{l}"] = bf(Wq_eff[:, 2 * D:])
        cb = b1 @ Wqkv                              # folded LN bias through qkv
        w[f"eqb{l}"] = pcol(cb[:D] * scale)         # q bias (per-partition)
        # k bias dropped (softmax row-invariant); v bias folded into proj bias
        w[f"eproj{l}"] = bf(g["enc_proj_w"][l])
        pb = g["enc_proj_b"][l] + cb[2 * D:] @ g["enc_proj_w"][l]
        w[f"epb{l}"] = pcol(pb)
        s2, b2 = g["enc_ln2_s"][l], g["enc_ln2_b"][l]
        W1 = g["enc_mlp_w1"][l]
        w[f"em1{l}"] = bf(s2[:, None] * W1)
        w[f"em1b{l}"] = pcol(b2 @ W1 + g["enc_mlp_b1"][l])
        w[f"em2{l}"] = bf(g["enc_mlp_w2"][l])
        w[f"em2b{l}"] = pcol(g["enc_mlp_b2"][l])

    w["e2dw"] = bf(g["e2d_w"])                      # (768, 512)
    w["e2db"] = pcol(g["e2d_b"])
    w["dq"] = np.ascontiguousarray(g["dec_query"][0].T.astype(f32))  # (512,128)

    dscale = HDD ** -0.5
    for l in range(LD):
        s1, b1 = g["dec_ln1_s"][l], g["dec_ln1_b"][l]
        Wq = g["dec_qkv_w"][l, 0]
        w[f"dwq{l}"] = bf(s1[:, None] * Wq * dscale)
        w[f"dqb{l}"] = pcol((b1 @ Wq + g["dec_qkv_b"][l, 0]) * dscale)
        w[f"dwk{l}"] = bf(g["dec_qkv_w"][l, 1])     # k bias dropped
        w[f"dwv{l}"] = bf(g["dec_qkv_w"][l, 2])
        w[f"dwo{l}"] = bf(g["dec_out_w"][l])
        ob = g["dec_out_b"][l] + g["dec_qkv_b"][l, 2] @ g["dec_out_w"][l]
        w[f"dob{l}"] = pcol(ob)
        s2, b2 = g["dec_ln2_s"][l], g["dec_ln2_b"][l]
        W1 = g["dec_mlp_w1"][l]
        w[f"dm1{l}"] = bf(s2[:, None] * W1)
        w[f"dm1b{l}"] = pcol(b2 @ W1 + g["dec_mlp_b1"][l])
        w[f"dm2{l}"] = bf(g["dec_mlp_w2"][l])
        w[f"dm2b{l}"] = pcol(g["dec_mlp_b2"][l])

    sh, bh = g["head_ln_s"], g["head_ln_b"]
    w["hw"] = bf(sh[:, None] * g["head_w"])         # (512, 768)
    hb = bh @ g["head_w"] + g["head_b"]             # (768,) per-FREE bias
    w["hbb"] = np.ascontiguousarray(
        np.broadcast_to(hb[None, :], (PP, OUT)).astype(f32))

    w["ident"] = np.ascontiguousarray(np.eye(PP, dtype=np.float32).astype(BF16))

    # block mask: allowed[l][qi] = tuple of allowed key-frame blocks
    mr = g["mask_rand"]                             # (L, T, T)
    allowed = []
    for l in range(L):
        per_q = []
        for i in range(T):
            ks = [j for j in range(T)
                  if j <= i or not (mr[l, i, j] < MASK_RATIO)]
            per_q.append(tuple(ks))
        allowed.append(per_q)

    return w, xfT_bf, allowed


# ------------------------------------------------------------------
# Tile tail-drain patch: this walrus build rejects >1 sync wait per
# instruction at the kernel-tail drain; split the waits across NOPs.
# ------------------------------------------------------------------

def _patch_tile():
    import concourse.tile as tile
    from concourse.vector_clock import ScopedClock, VectorClock

    if getattr(tile.TileContext, "_drain_patched", False):
        return

    def _drain_and_barrier_chunked(self, tick_clock, wait_clock):
        g = list(tick_clock.global_clock)
        procs = [i for i, v in enumerate(g) if v > 0]
        for p in procs:
            sub = [0] * len(g)
            sub[p] = g[p]
            nop_inst = self.nc.sync.nop(nofuse=True)
            wait_clock.add_sem_waits(
                nop_inst.ins, ScopedClock({None: VectorClock(sub)}))
        self.nc.sync.drain()
        self.nc.all_engine_barrier()
        assert self.sems is not None
        popped = self.nc._tile_sem_poison_stack.pop()
        assert popped is self._sem_poison
        self.nc.clear_and_free_semaphores(list(self.sems.allocated().values()))
        self.nc.all_engine_barrier()

    tile.TileContext._drain_and_barrier = _drain_and_barrier_chunked

    # This walrus build also rejects >1 sync wait on regular engine
    # instructions (Matmult etc.).  Hoist excess waits onto same-engine
    # NOPs inserted immediately before the instruction.
    from concourse import mybir as _mybir

    _orig_lower = tile.TileContext._lower_ordered_insts

    def _split_waits_and_lower(self, ordered):
        nctr = [0]
        for bb_name, insts in ordered.items():
            new_list = []
            for inst in insts:
                si = getattr(inst, "sync_info", None)
                waits = list(si.on_wait) if si is not None else []
                if len(waits) > 1:
                    imm = [w for w in waits if w.wait_reg is None]
                    reg = [w for w in waits if w.wait_reg is not None]
                    keep = imm[:1] + reg  # keep one imm (plus any reg waits)
                    excess = imm[1:]
                    for w in excess:
                        nctr[0] += 1
                        nop = _mybir.InstNoOp(
                            name=f"{inst.name}-wsplit{nctr[0]}", ins=[], outs=[])
                        nop.engine = inst.engine
                        nop.sync_info = _mybir.SyncInfo(
                            on_wait=[w], on_update=[])
                        self.nc.register_instruction(nop, overwrite=True)
                        new_list.append(nop)
                    inst.sync_info = _mybir.SyncInfo(
                        on_wait=keep, on_update=list(si.on_update))
                new_list.append(inst)
            insts[:] = new_list
        return _orig_lower(self, ordered)

    tile.TileContext._lower_ordered_insts = _split_waits_and_lower
    tile.TileContext._drain_patched = True


# ------------------------------------------------------------------
# graph builder
# ------------------------------------------------------------------

def _build(allowed, dbg=()):
    import concourse.bass as bass
    import concourse.tile as tile
    from concourse import mybir

    _patch_tile()
    f32 = mybir.dt.float32
    bf16 = mybir.dt.bfloat16
    AF = mybir.ActivationFunctionType
    OP = mybir.AluOpType

    nc = bass.Bass()

    # ---- DRAM parameters ----
    dp = {}

    def din(name, shape, dtype):
        dp[name] = nc.declare_dram_parameter(name, list(shape), dtype, isOutput=False)
        return dp[name]

    din("xfT", (24 * PP, S), bf16)
    din("wk", (24 * PP, D), bf16)
    din("pos", (D, S), f32)
    for l in range(L):
        din(f"eqk{l}", (D, 2 * D), bf16)
        din(f"evw{l}", (D, D), bf16)
        din(f"eqb{l}", (PP, 6), f32)
        din(f"eproj{l}", (D, D), bf16)
        din(f"epb{l}", (PP, 6), f32)
        din(f"em1{l}", (D, MLP), bf16)
        din(f"em1b{l}", (PP, 24), f32)
        din(f"em2{l}", (MLP, D), bf16)
        din(f"em2b{l}", (PP, 6), f32)
    din("e2dw", (D, DD), bf16)
    din("e2db", (PP, 4), f32)
    din("dq", (DD, P), f32)
    for l in range(LD):
        din(f"dwq{l}", (DD, DD), bf16)
        din(f"dqb{l}", (PP, 4), f32)
        din(f"dwk{l}", (DD, DD), bf16)
        din(f"dwv{l}", (DD, DD), bf16)
        din(f"dwo{l}", (DD, DD), bf16)
        din(f"dob{l}", (PP, 4), f32)
        din(f"dm1{l}", (DD, MLPD), bf16)
        din(f"dm1b{l}", (PP, 16), f32)
        din(f"dm2{l}", (MLPD, DD), bf16)
        din(f"dm2b{l}", (PP, 4), f32)
    din("hw", (DD, OUT), bf16)
    din("ident", (PP, PP), bf16)
    din("hbb", (PP, OUT), f32)
    out_ext = nc.declare_dram_parameter("out", [P, OUT], f32, isOutput=True)
    dbg_ext = {name: nc.declare_dram_parameter(name, [PP, 6, S], f32, isOutput=True)
               for name in dbg}

    with tile.TileContext(nc) as tc:
        with (
            tc.tile_pool(name="consts", bufs=1) as consts,
            tc.tile_pool(name="wpool", bufs=4) as wpool,
            tc.tile_pool(name="bias", bufs=6) as biasp,
            tc.tile_pool(name="act", bufs=1) as act,
            tc.tile_pool(name="tmp", bufs=2) as tmp,
            tc.tile_pool(name="hsqp", bufs=1) as hsqp,
            tc.tile_pool(name="bigp", bufs=1) as bigp,
            tc.tile_pool(name="attn", bufs=10) as attnp,
            tc.tile_pool(name="rrsp", bufs=2) as rrsp,
            tc.tile_pool(name="small", bufs=2) as small,
            tc.tile_pool(name="tiny", bufs=8) as tiny,
            tc.tile_pool(name="pp_mm", bufs=3, space="PSUM") as pp_mm,
            tc.tile_pool(name="pp_sc", bufs=2, space="PSUM") as pp_sc,
            tc.tile_pool(name="pp_pv", bufs=1, space="PSUM") as pp_pv,
            tc.tile_pool(name="pp_st", bufs=1, space="PSUM") as pp_st,
            tc.tile_pool(name="pp_bc", bufs=1, space="PSUM") as pp_bc,
        ):
            ones_f32 = consts.tile([PP, 1], f32)
            nc.vector.memset(ones_f32, 1.0)
            ones_row = consts.tile([1, PP], f32)
            nc.vector.memset(ones_row, 1.0)
            ones_bf16 = consts.tile([PP, 1], bf16)
            nc.vector.memset(ones_bf16, 1.0)
            ones_row_bf = consts.tile([1, PP], bf16)
            nc.vector.memset(ones_row_bf, 1.0)
            eps_t = consts.tile([1, 1], f32)
            nc.vector.memset(eps_t, EPS)

            def load_w(name, KO, M, dtype=bf16, tag="w"):
                t = wpool.tile([PP, KO, M], dtype, tag=tag)
                nc.sync.dma_start(
                    out=t, in_=dp[name][:].rearrange("(ko p) m -> p ko m", p=PP))
                return t

            W_SLOT = 6144  # bf16 elems per partition in a weight slot

            def load_b(name, MO):
                t = biasp.tile([PP, MO], f32, tag="b")
                nc.sync.dma_start(out=t, in_=dp[name][:])
                return t

            # dense matmul with chunked weight streaming from DRAM.
            # out feature-major; rhs (128, KO, N); evac(m, psum)
            def dense(wname, KO, MO, rhs_sb, N, evac):
                M = MO * PP
                mch_cols = max(PP, (W_SLOT // KO) // PP * PP)
                wap = dp[wname][:].rearrange("(ko p) m -> p ko m", p=PP)
                for c0 in range(0, M, mch_cols):
                    mch = min(mch_cols, M - c0)
                    wt = wpool.tile([PP, KO, mch], bf16, tag="w")
                    nc.sync.dma_start(out=wt, in_=wap[:, :, c0:c0 + mch])
                    for mi in range(mch // PP):
                        m = c0 // PP + mi
                        ps = pp_mm.tile([PP, 512], f32, tag="mm")
                        for k in range(KO):
                            nc.tensor.matmul(
                                ps[:, :N],
                                lhsT=wt[:, k, mi * PP:(mi + 1) * PP],
                                rhs=rhs_sb[:, k, :],
                                start=(k == 0), stop=(k == KO - 1))
                        evac(m, ps[:, :N])

            # layer norm, feature-major input (128, KO, W) f32 -> bf16 out
            def lnorm(h_sb, KO, Wd, y_sb):
                Dm = KO * PP
                hb = hsqp.tile([PP, KO, Wd], bf16, tag="hb")
                hsq = hsqp.tile([PP, KO, Wd], bf16, tag="hsq")
                st = pp_st.tile([33, Wd], f32, tag="st")
                for k in range(KO):
                    nc.vector.tensor_copy(hb[:, k, :], h_sb[:, k, :])
                    nc.tensor.matmul(st[0:1, :], lhsT=ones_bf16, rhs=hb[:, k, :],
                                     start=(k == 0), stop=(k == KO - 1))
                    nc.scalar.activation(hsq[:, k, :], h_sb[:, k, :], AF.Square)
                    nc.tensor.matmul(st[32:33, :], lhsT=ones_bf16, rhs=hsq[:, k, :],
                                     start=(k == 0), stop=(k == KO - 1))
                mean = small.tile([1, Wd], f32, tag="s1")
                nc.vector.tensor_scalar_mul(mean, st[0:1, :], 1.0 / Dm)
                var = small.tile([1, Wd], f32, tag="s2")
                nc.vector.tensor_scalar_mul(var, st[32:33, :], 1.0 / Dm)
                msq = small.tile([1, Wd], f32, tag="s3")
                nc.vector.tensor_mul(msq, mean, mean)
                nc.vector.tensor_sub(var, var, msq)
                # 1/sqrt(var+eps) = exp(-0.5*ln(var+eps)): stays in the
                # exp/ln ACT table set (no sqrt-set switch, no slow DVE
                # reciprocal)
                nc.scalar.activation(var, var, AF.Ln, bias=eps_t)
                inv = small.tile([1, Wd], f32, tag="s5")
                nc.scalar.activation(inv, var, AF.Exp, scale=-0.5)
                mean_bf = small.tile([1, Wd], bf16, tag="s6")
                nc.vector.tensor_copy(mean_bf, mean)
                inv_bf = small.tile([1, Wd], bf16, tag="s7")
                nc.vector.tensor_copy(inv_bf, inv)
                mb = pp_bc.tile([PP, Wd], f32, tag="bc")
                nc.tensor.matmul(mb, lhsT=ones_row_bf, rhs=mean_bf,
                                 start=True, stop=True)
                for k in range(KO):
                    nc.vector.tensor_tensor(
                        y_sb[:, k, :], h_sb[:, k, :], mb, OP.subtract)
                ib = pp_bc.tile([PP, Wd], f32, tag="bc")
                nc.tensor.matmul(ib, lhsT=ones_row_bf, rhs=inv_bf,
                                 start=True, stop=True)
                for k in range(KO):
                    nc.vector.tensor_tensor(
                        y_sb[:, k, :], y_sb[:, k, :], ib, OP.mult)

            # attention, transposed-scores formulation: no p transposes.
            # The two heads of a 128-partition pair are interleaved matmul-by-
            # matmul so they land on disjoint PE row/col groups and execute
            # concurrently.  Row sums for both heads share one PSUM tile at
            # partitions 0 and 32.
            def attention(q_sb, k_sb, vT_sb, o_sb, n_heads, n_q_tiles,
                          allowed_per_qi, v_fill=None):
                kj_all = sorted({kj for qi in range(n_q_tiles)
                                 for kj in allowed_per_qi[qi]})
                kj_to_qi = {kj: [qi for qi in range(n_q_tiles)
                                 if kj in allowed_per_qi[qi]] for kj in kj_all}

                def qi_runs(qis):
                    runs = []
                    i = 0
                    while i < len(qis):
                        j = i
                        while j + 1 < len(qis) and qis[j + 1] == qis[j] + 1:
                            j += 1
                        runs.append(qis[i:j + 1])
                        i = j + 1
                    return runs

                W0 = n_q_tiles * PP
                # pack consecutive key blocks into <=512-col score tiles so
                # each ScalarE exp covers more columns (fixed ~352cyc op
                # overhead amortizes)
                if n_q_tiles == 1:
                    packs = []
                    cur = []
                    cur_cols = 0
                    for kj in kj_all:
                        nc_kj = len(kj_to_qi[kj]) * PP
                        if cur and cur_cols + nc_kj > 512:
                            packs.append(cur)
                            cur, cur_cols = [], 0
                        cur.append(kj)
                        cur_cols += nc_kj
                    if cur:
                        packs.append(cur)
                else:
                    packs = [[kj] for kj in kj_all]
                for pair in range(n_heads // 2):
                    if v_fill is not None:
                        v_fill(pair)   # dense filler work: keeps PE warm
                    pt2 = [{}, {}]
                    # scores + exp, subs interleaved per key block
                    for pack in packs:
                        pcols = sum(len(kj_to_qi[kj]) for kj in pack) * PP
                        sc2 = []
                        for sub in range(2):
                            sc2.append(pp_sc.tile([PP, 512], f32, tag="sc",
                                                  name=f"sc{sub}"))
                        base = 0
                        for kj in pack:
                            qis = kj_to_qi[kj]
                            for run in qi_runs(qis):
                                col = base + qis.index(run[0]) * PP
                                for sub in range(2):
                                    b0 = 64 * sub
                                    nc.tensor.matmul(
                                        sc2[sub][:, col:col + len(run) * PP],
                                        lhsT=k_sb[b0:b0 + 64, pair,
                                                  kj * PP:(kj + 1) * PP],
                                        rhs=q_sb[b0:b0 + 64, pair,
                                                 run[0] * PP:
                                                 (run[-1] + 1) * PP],
                                        start=True, stop=True)
                            base += len(qis) * PP
                        for sub in range(2):
                            pt = attnp.tile([PP, 512], bf16, tag="p")
                            nc.scalar.activation(pt[:, :pcols],
                                                 sc2[sub][:, :pcols], AF.Exp)
                            base = 0
                            for kj in pack:
                                qis = kj_to_qi[kj]
                                pt2[sub][kj] = (pt, {qi: base + i * PP
                                                     for i, qi
                                                     in enumerate(qis)})
                                base += len(qis) * PP
                    # row sums: separate tiles per sub (same-bank PE-write +
                    # ACT-read on disjoint partitions is a HW fault)
                    rsps2 = [pp_st.tile([1, 512], f32, tag="st", name="rs0"),
                             pp_st.tile([1, 512], f32, tag="st", name="rs1")]
                    seen = [[0] * n_q_tiles, [0] * n_q_tiles]
                    nkj = {qi: len(allowed_per_qi[qi])
                           for qi in range(n_q_tiles)}
                    for kj in kj_all:
                        qis = kj_to_qi[kj]
                        for run in qi_runs(qis):
                            for sub in range(2):
                                pt, cols = pt2[sub][kj]
                                nc.tensor.matmul(
                                    rsps2[sub][0:1,
                                               run[0] * PP:(run[-1] + 1) * PP],
                                    lhsT=ones_bf16,
                                    rhs=pt[:, cols[run[0]]:
                                           cols[run[0]] + len(run) * PP],
                                    start=(seen[sub][run[0]] == 0),
                                    stop=(seen[sub][run[0]]
                                          == nkj[run[0]] - 1))
                            for qi in run:
                                seen[0][qi] += 1
                                seen[1][qi] += 1
                    # 1/rowsum via exp(-ln(x)) on ScalarE, both subs
                    rr2 = []
                    for sub in range(2):
                        rr = small.tile([1, 512], f32, tag="rr", name=f"rrx{sub}")
                        lnr = small.tile([1, 512], f32, tag="s3")
                        nc.scalar.activation(lnr[:, :W0],
                                             rsps2[sub][:, :W0], AF.Ln)
                        nc.scalar.activation(rr[:, :W0], lnr[:, :W0],
                                             AF.Exp, scale=-1.0)
                        rr2.append(rr)
                    rrb = pp_bc.tile([PP, 512], f32, tag="bc")
                    nc.tensor.matmul(rrb[0:64, :W0], lhsT=ones_row[:, :64],
                                     rhs=rr2[0][:, :W0], start=True, stop=True)
                    nc.tensor.matmul(rrb[64:128, :W0], lhsT=ones_row[:, :64],
                                     rhs=rr2[1][:, :W0], start=True, stop=True)
                    rrs = rrsp.tile([PP, 512], bf16, tag="rrs")
                    nc.vector.tensor_copy(rrs[:, :W0], rrb[:, :W0])
                    # PV, subs interleaved, merged over contiguous qi runs
                    po_ps = pp_pv.tile([PP, 512], f32, tag="pv")
                    for kj in kj_all:
                        qis = kj_to_qi[kj]
                        i = 0
                        while i < len(qis):
                            qi0 = qis[i]
                            st0 = (kj == allowed_per_qi[qi0][0])
                            sp0 = (kj == allowed_per_qi[qi0][-1])
                            j = i
                            while (j + 1 < len(qis)
                                   and qis[j + 1] == qis[j] + 1
                                   and (kj == allowed_per_qi[
                                       qis[j + 1]][0]) == st0
                                   and (kj == allowed_per_qi[
                                       qis[j + 1]][-1]) == sp0):
                                j += 1
                            run = qis[i:j + 1]
                            for s2 in range(2):
                                hh = 2 * pair + s2
                                pt, cols = pt2[s2][kj]
                                nc.tensor.matmul(
                                    po_ps[64 * s2:64 * s2 + 64,
                                          run[0] * PP:(run[-1] + 1) * PP],
                                    lhsT=vT_sb[:, kj, hh * 64:(hh + 1) * 64],
                                    rhs=pt[:, cols[run[0]]:
                                           cols[run[0]] + len(run) * PP],
                                    start=st0, stop=sp0)
                            i = j + 1
                    nc.vector.tensor_tensor(
                        o_sb[:, pair, :W0], po_ps[:, :W0],
                        rrs[:, :W0], OP.mult)

            # ---------------- patch embedding ----------------
            xf_sb = bigp.tile([PP, 24, S], bf16, tag="big")
            nc.sync.dma_start(
                out=xf_sb, in_=dp["xfT"][:].rearrange("(ko p) m -> p ko m", p=PP))
            pos_sb = act.tile([PP, 6, S], f32)
            nc.sync.dma_start(
                out=pos_sb, in_=dp["pos"][:].rearrange("(ko p) m -> p ko m", p=PP))
            h_sb = act.tile([PP, 6, S], f32)

            def embed_evac(m, ps):
                nc.vector.tensor_tensor(h_sb[:, m, :], ps, pos_sb[:, m, :],
                                        OP.add)
            dense("wk", 24, 6, xf_sb, S, embed_evac)

            if "dbg_h0" in dbg_ext:
                nc.sync.dma_start(out=dbg_ext["dbg_h0"][:], in_=h_sb)

            # ---------------- encoder layers ----------------
            y_sb = act.tile([PP, 6, S], bf16)
            q_sb = act.tile([PP, 6, S], bf16)
            k_sb = act.tile([PP, 6, S], bf16)
            vT_sb = act.tile([PP, 4, D], bf16)
            o_sb = act.tile([PP, 6, S], bf16)
            for l in range(L):
                lnorm(h_sb, 6, S, y_sb)
                qb = load_b(f"eqb{l}", 6)

                def qkv_evac(m, ps):
                    if m < 6:      # Q with bias
                        nc.vector.tensor_scalar_add(q_sb[:, m, :], ps,
                                                    qb[:, m:m + 1])
                    else:          # K plain
                        nc.vector.tensor_copy(k_sb[:, m - 6, :], ps)
                dense(f"eqk{l}", 6, 12, y_sb, S, qkv_evac)
                # V token-major, computed per head-pair column slice inside
                # the attention loop (fills PE while ScalarE runs exps)
                wv = load_w(f"evw{l}", 6, D)

                def v_fill(pair):
                    pcs = slice(pair * PP, (pair + 1) * PP)
                    for jb in range(4):
                        ps = pp_mm.tile([PP, 512], f32, tag="mm")
                        for k in range(6):
                            nc.tensor.matmul(
                                ps[:, :PP],
                                lhsT=y_sb[:, k, jb * PP:(jb + 1) * PP],
                                rhs=wv[:, k, pcs],
                                start=(k == 0), stop=(k == 5))
                        nc.vector.tensor_copy(vT_sb[:, jb, pcs], ps[:, :PP])

                attention(q_sb, k_sb, vT_sb, o_sb, NH, 4, allowed[l],
                          v_fill=v_fill)

                pb = load_b(f"epb{l}", 6)

                def proj_evac(m, ps):
                    t = tmp.tile([PP, S], f32, tag="ev")
                    nc.vector.tensor_scalar_add(t, ps, pb[:, m:m + 1])
                    nc.gpsimd.tensor_tensor(h_sb[:, m, :], h_sb[:, m, :], t,
                                            OP.add)
                dense(f"eproj{l}", 6, 6, o_sb, S, proj_evac)

                lnorm(h_sb, 6, S, y_sb)
                g_sb = bigp.tile([PP, 24, S], bf16, tag="big")
                m1b = load_b(f"em1b{l}", 24)

                def gelu_evac(m, ps):
                    nc.scalar.activation(g_sb[:, m, :], ps, AF.Gelu,
                                         bias=m1b[:, m:m + 1])
                dense(f"em1{l}", 6, 24, y_sb, S, gelu_evac)

                m2b = load_b(f"em2b{l}", 6)

                def mlp2_evac(m, ps):
                    t = tmp.tile([PP, S], f32, tag="ev")
                    nc.vector.tensor_scalar_add(t, ps, m2b[:, m:m + 1])
                    nc.gpsimd.tensor_tensor(h_sb[:, m, :], h_sb[:, m, :], t,
                                            OP.add)
                dense(f"em2{l}", 24, 6, g_sb, S, mlp2_evac)

                if f"dbg_he{l}" in dbg_ext:
                    nc.sync.dma_start(out=dbg_ext[f"dbg_he{l}"][:], in_=h_sb)

            # ---------------- encoder -> decoder ----------------
            nc.vector.tensor_copy(y_sb, h_sb)
            e2db = load_b("e2db", 4)
            memT_sb = act.tile([PP, 4, S], bf16)   # feature-major mem

            def e2d_evac(m, ps):
                nc.vector.tensor_scalar_add(memT_sb[:, m, :], ps,
                                            e2db[:, m:m + 1])
            dense("e2dw", 6, 4, y_sb, S, e2d_evac)

            # ---------------- decoder ----------------
            qd_sb = act.tile([PP, 4, P], f32)      # decoder residual stream
            nc.sync.dma_start(
                out=qd_sb, in_=dp["dq"][:].rearrange("(ko p) m -> p ko m", p=PP))

            yd_sb = act.tile([PP, 4, P], bf16)
            Qd_sb = act.tile([PP, 4, P], bf16)
            Kd_sb = act.tile([PP, 4, S], bf16)
            vTd_sb = act.tile([PP, 4, DD], bf16)
            od_sb = act.tile([PP, 4, P], bf16)
            gd_sb = act.tile([PP, 16, P], bf16)

            for l in range(LD):

                def kd_evac(m, ps):
                    nc.vector.tensor_copy(Kd_sb[:, m, :], ps)
                dense(f"dwk{l}", 4, 4, memT_sb, S, kd_evac)

                wvd = load_w(f"dwv{l}", 4, DD)
                for jb in range(4):
                    ps = pp_mm.tile([PP, 512], f32, tag="mm")
                    for k in range(4):
                        nc.tensor.matmul(
                            ps[:, :DD],
                            lhsT=memT_sb[:, k, jb * PP:(jb + 1) * PP],
                            rhs=wvd[:, k, :],
                            start=(k == 0), stop=(k == 3))
                    nc.vector.tensor_copy(vTd_sb[:, jb, :], ps[:, :DD])

                lnorm(qd_sb, 4, P, yd_sb)
                qbd = load_b(f"dqb{l}", 4)

                def qd_evac(m, ps):
                    nc.vector.tensor_scalar_add(Qd_sb[:, m, :], ps,
                                                qbd[:, m:m + 1])
                dense(f"dwq{l}", 4, 4, yd_sb, P, qd_evac)

                attention(Qd_sb, Kd_sb, vTd_sb, od_sb, NHD, 1,
                          [(0, 1, 2, 3)])

                obd = load_b(f"dob{l}", 4)

                def od_evac(m, ps):
                    t = tmp.tile([PP, S], f32, tag="ev")
                    nc.vector.tensor_scalar_add(t[:, :P], ps, obd[:, m:m + 1])
                    nc.gpsimd.tensor_tensor(qd_sb[:, m, :], qd_sb[:, m, :],
                                            t[:, :P], OP.add)
                dense(f"dwo{l}", 4, 4, od_sb, P, od_evac)

                lnorm(qd_sb, 4, P, yd_sb)
                m1bd = load_b(f"dm1b{l}", 16)

                def gelud_evac(m, ps):
                    nc.scalar.activation(gd_sb[:, m, :], ps, AF.Gelu,
                                         bias=m1bd[:, m:m + 1])
                dense(f"dm1{l}", 4, 16, yd_sb, P, gelud_evac)

                m2bd = load_b(f"dm2b{l}", 4)

                def mlp2d_evac(m, ps):
                    t = tmp.tile([PP, S], f32, tag="ev")
                    nc.vector.tensor_scalar_add(t[:, :P], ps, m2bd[:, m:m + 1])
                    nc.gpsimd.tensor_tensor(qd_sb[:, m, :], qd_sb[:, m, :],
                                            t[:, :P], OP.add)
                dense(f"dm2{l}", 16, 4, gd_sb, P, mlp2d_evac)

            # ---------------- head ----------------
            lnorm(qd_sb, 4, P, yd_sb)
            wh = load_w("hw", 4, OUT)
            hbb_sb = act.tile([PP, OUT], f32)
            nc.sync.dma_start(out=hbb_sb, in_=dp["hbb"][:])
            out_sb = act.tile([P, OUT], f32)
            for nchunk in range(2):
                ncs = slice(nchunk * 384, (nchunk + 1) * 384)
                ps = pp_mm.tile([PP, 512], f32, tag="mm")
                for k in range(4):
                    nc.tensor.matmul(ps[:, :384],
                                     lhsT=yd_sb[:, k, :],
                                     rhs=wh[:, k, ncs],
                                     start=(k == 0), stop=(k == 3))
                nc.vector.tensor_tensor(out_sb[:, ncs], ps[:, :384],
                                        hbb_sb[:, ncs], OP.add)
            nc.sync.dma_start(out=out_ext[:], in_=out_sb)

    return nc


# ------------------------------------------------------------------
# entry point
# ------------------------------------------------------------------

def kernel(dbg=(), _trace=False, _tmpdir=None, _full=False, **inputs):
    from concourse.bass_utils import run_bass_kernel_spmd

    w, xfT_bf, allowed = _prep(inputs)
    nc = _build(allowed, dbg=dbg)
    in_maps = []
    for b in range(B):
        m = dict(w)
        m["xfT"] = xfT_bf[b]
        in_maps.append(m)
    res = run_bass_kernel_spmd(nc, in_maps, core_ids=list(range(8)),
                               trace=_trace, tmpdir=_tmpdir)
    out = np.stack([np.asarray(res.results[i]["out"]) for i in range(B)])
    if dbg or _full:
        dbgs = {name: np.stack([np.asarray(res.results[i][name])
                                for i in range(B)]) for name in dbg}
        return out.astype(np.float32), dbgs, res
    return out.astype(np.float32)



# revision 8
# speedup vs baseline: 1.1999x; 1.1999x over previous
"""Trainium2 Bass kernel for nn_ARPredVideoVanilla (8-core data-parallel).

Strategy: pure data parallelism over batch (B=8 -> 1 batch element per core,
no collectives).  Activations live feature-major in SBUF.  Key optimizations
over the bf16 baseline:

- fp8(e4m3) DoubleRow matmuls (2 contraction rows per PE cell, ~1.44x) for the
  quantization-tolerant GEMMs: patch embed, Q/K projection, MLP fc1.  Weights
  are scaled x64 into the e4m3 normal range; the 1/64 is folded into the
  PSUM-evacuation ops.
- LayerNorm mean-subtraction is folded into the next matmul by column-centering
  its weights on the host (exact identity); the 1/std multiply is deferred to
  the PSUM evacuation.  The LN serial chain no longer blocks the PE.
- Softmax row sums come for free from an appended ones-column in V (the PV
  matmul's 65th output row); reciprocals are computed batched (ln+exp over all
  head rowsums at once, on parallel partitions).
- ACT table-set switches (natural_log_exp <-> gelu) are prewarmed with dummy
  activations so the ~2.7us loads overlap matmul phases.
"""

import sys

sys.path.insert(0, "/opt/trn_rl_repo")

import numpy as np
import ml_dtypes

BF16 = ml_dtypes.bfloat16
F8 = ml_dtypes.float8_e4m3
W8SCALE = 64.0

# ---- model dims (hardcoded from the problem spec) ----
B, T, V = 8, 4, 3
C = V * T                      # 12
H, W, PH, PW = 128, 256, 16, 16
HP, WP = H // PH, W // PW      # 8, 16
P = HP * WP                    # 128 patches/frame
S = T * P                      # 512
D, NH, HD, L = 768, 12, 64, 8
DD, NHD, HDD, LD = 512, 8, 64, 4
MLP, MLPD = 3072, 2048
OUT = PH * PW * V              # 768
MASK_RATIO = 0.8
EPS = 1e-5
PP = 128  # partitions


# ------------------------------------------------------------------
# host-side preparation: fold biases/scales, center, transpose, cast
# ------------------------------------------------------------------

def _prep(inputs):
    f32 = np.float32
    g = {k: np.asarray(v, dtype=f32) for k, v in inputs.items()}

    w = {}

    def bf(a):
        return np.ascontiguousarray(a.astype(BF16))

    def f8w(a):  # fp8 weight with x64 scaling
        return np.ascontiguousarray((a * W8SCALE).astype(F8))

    def cc(a):  # column-center (folds LN mean subtraction)
        return a - a.mean(axis=0, keepdims=True)

    def pcol(bias):  # (M,) -> (128, M//128) per-partition layout, tile-major
        M = bias.shape[0]
        return np.ascontiguousarray(bias.reshape(M // PP, PP).T.astype(f32))

    # patch data, per core: x[b] (T,C,H,W) -> xfT (C*PH*PW, T*P) in fp8
    x = g["x"]  # (B,T,C,H,W)
    xf = x.reshape(B, T, C, HP, PH, WP, PW).transpose(0, 1, 3, 5, 2, 4, 6)
    xf = xf.reshape(B, T * P, C * PH * PW)          # (B, 512, 3072)
    xfT = np.ascontiguousarray(np.swapaxes(xf, 1, 2))  # (B, 3072, 512)
    xfT_f8 = [np.ascontiguousarray(xfT[b].astype(F8)) for b in range(B)]

    # conv: wk (3072, 768) fp8 x64; pos_eff (768, 512) f32 with conv_b folded
    wk = g["conv_w"].reshape(D, C * PH * PW).T      # (3072, 768)
    w["wk"] = f8w(wk)
    pos = g["pos_emb"][0].T + g["conv_b"][:, None]  # (768, 512)
    w["pos"] = np.ascontiguousarray(pos.astype(f32))

    scale = HD ** -0.5
    for l in range(L):
        s1, b1 = g["enc_ln1_s"][l], g["enc_ln1_b"][l]
        Wqkv = g["enc_qkv_w"][l]                    # (768, 2304)
        Ws = s1[:, None] * Wqkv
        cb = b1 @ Wqkv                              # LN bias folded through qkv
        Wc = cc(Ws)                                 # center (mean-sub fold)
        Wqk = Wc[:, :2 * D].copy()
        Wqk[:, :D] *= scale
        w[f"eqk{l}"] = f8w(Wqk)                     # fp8 x64
        w[f"evw{l}"] = bf(Wc[:, 2 * D:])            # V bf16
        w[f"eqb{l}"] = pcol(cb[:D] * scale)         # q bias (per-partition)
        # k bias dropped (softmax row-invariant); v bias folded into proj bias
        w[f"eproj{l}"] = bf(g["enc_proj_w"][l])
        pb = g["enc_proj_b"][l] + cb[2 * D:] @ g["enc_proj_w"][l]
        w[f"epb{l}"] = pcol(pb)
        s2, b2 = g["enc_ln2_s"][l], g["enc_ln2_b"][l]
        W1 = g["enc_mlp_w1"][l]
        w[f"em1{l}"] = f8w(cc(s2[:, None] * W1))    # fp8 x64 centered
        w[f"em1b{l}"] = pcol(b2 @ W1 + g["enc_mlp_b1"][l])
        w[f"em2{l}"] = bf(g["enc_mlp_w2"][l])
        w[f"em2b{l}"] = pcol(g["enc_mlp_b2"][l])

    w["e2dw"] = bf(g["e2d_w"])                      # (768, 512)
    w["e2db"] = pcol(g["e2d_b"])
    w["dq"] = np.ascontiguousarray(g["dec_query"][0].T.astype(f32))  # (512,128)

    dscale = HDD ** -0.5
    for l in range(LD):
        s1, b1 = g["dec_ln1_s"][l], g["dec_ln1_b"][l]
        Wq = g["dec_qkv_w"][l, 0]
        w[f"dwq{l}"] = bf(cc(s1[:, None] * Wq) * dscale)
        w[f"dqb{l}"] = pcol((b1 @ Wq + g["dec_qkv_b"][l, 0]) * dscale)
        w[f"dwk{l}"] = bf(g["dec_qkv_w"][l, 1])     # k bias dropped
        w[f"dwv{l}"] = bf(g["dec_qkv_w"][l, 2])
        w[f"dwo{l}"] = bf(g["dec_out_w"][l])
        ob = g["dec_out_b"][l] + g["dec_qkv_b"][l, 2] @ g["dec_out_w"][l]
        w[f"dob{l}"] = pcol(ob)
        s2, b2 = g["dec_ln2_s"][l], g["dec_ln2_b"][l]
        W1 = g["dec_mlp_w1"][l]
        w[f"dm1{l}"] = bf(cc(s2[:, None] * W1))
        w[f"dm1b{l}"] = pcol(b2 @ W1 + g["dec_mlp_b1"][l])
        w[f"dm2{l}"] = bf(g["dec_mlp_w2"][l])
        w[f"dm2b{l}"] = pcol(g["dec_mlp_b2"][l])

    sh, bh = g["head_ln_s"], g["head_ln_b"]
    w["hw"] = bf(cc(sh[:, None] * g["head_w"]))     # (512, 768) centered
    hb = bh @ g["head_w"] + g["head_b"]             # (768,) per-FREE bias
    w["hbb"] = np.ascontiguousarray(
        np.broadcast_to(hb[None, :], (PP, OUT)).astype(f32))

    # block mask: allowed[l][qi] = tuple of allowed key-frame blocks
    mr = g["mask_rand"]                             # (L, T, T)
    allowed = []
    for l in range(L):
        per_q = []
        for i in range(T):
            ks = [j for j in range(T)
                  if j <= i or not (mr[l, i, j] < MASK_RATIO)]
            per_q.append(tuple(ks))
        allowed.append(per_q)

    return w, xfT_f8, allowed


# ------------------------------------------------------------------
# Tile tail-drain patch: this walrus build rejects >1 sync wait per
# instruction at the kernel-tail drain; split the waits across NOPs.
# ------------------------------------------------------------------

def _patch_tile():
    import concourse.tile as tile
    from concourse.vector_clock import ScopedClock, VectorClock

    if getattr(tile.TileContext, "_drain_patched", False):
        return

    def _drain_and_barrier_chunked(self, tick_clock, wait_clock):
        g = list(tick_clock.global_clock)
        procs = [i for i, v in enumerate(g) if v > 0]
        for p in procs:
            sub = [0] * len(g)
            sub[p] = g[p]
            nop_inst = self.nc.sync.nop(nofuse=True)
            wait_clock.add_sem_waits(
                nop_inst.ins, ScopedClock({None: VectorClock(sub)}))
        self.nc.sync.drain()
        self.nc.all_engine_barrier()
        assert self.sems is not None
        popped = self.nc._tile_sem_poison_stack.pop()
        assert popped is self._sem_poison
        self.nc.clear_and_free_semaphores(list(self.sems.allocated().values()))
        self.nc.all_engine_barrier()

    tile.TileContext._drain_and_barrier = _drain_and_barrier_chunked

    # This walrus build also rejects >1 sync wait on regular engine
    # instructions (Matmult etc.).  Hoist excess waits onto same-engine
    # NOPs inserted immediately before the instruction.
    from concourse import mybir as _mybir

    _orig_lower = tile.TileContext._lower_ordered_insts

    def _split_waits_and_lower(self, ordered):
        nctr = [0]
        for bb_name, insts in ordered.items():
            new_list = []
            for inst in insts:
                si = getattr(inst, "sync_info", None)
                waits = list(si.on_wait) if si is not None else []
                if len(waits) > 1:
                    imm = [w for w in waits if w.wait_reg is None]
                    reg = [w for w in waits if w.wait_reg is not None]
                    keep = imm[:1] + reg  # keep one imm (plus any reg waits)
                    excess = imm[1:]
                    for w in excess:
                        nctr[0] += 1
                        nop = _mybir.InstNoOp(
                            name=f"{inst.name}-wsplit{nctr[0]}", ins=[], outs=[])
                        nop.engine = inst.engine
                        nop.sync_info = _mybir.SyncInfo(
                            on_wait=[w], on_update=[])
                        self.nc.register_instruction(nop, overwrite=True)
                        new_list.append(nop)
                    inst.sync_info = _mybir.SyncInfo(
                        on_wait=keep, on_update=list(si.on_update))
                new_list.append(inst)
            insts[:] = new_list
        return _orig_lower(self, ordered)

    tile.TileContext._lower_ordered_insts = _split_waits_and_lower
    tile.TileContext._drain_patched = True


# ------------------------------------------------------------------
# graph builder
# ------------------------------------------------------------------

def _build(allowed, dbg=()):
    import concourse.bass as bass
    import concourse.tile as tile
    from concourse import mybir

    _patch_tile()
    f32 = mybir.dt.float32
    bf16 = mybir.dt.bfloat16
    fp8 = mybir.dt.float8e4
    AF = mybir.ActivationFunctionType
    OP = mybir.AluOpType
    DR = mybir.MatmulPerfMode.DoubleRow
    INV_W8 = 1.0 / W8SCALE
    LN_W8 = float(np.log(1.0 / W8SCALE))

    nc = bass.Bass()

    # ---- DRAM parameters ----
    dp = {}

    def din(name, shape, dtype):
        dp[name] = nc.declare_dram_parameter(name, list(shape), dtype, isOutput=False)
        return dp[name]

    din("xfT", (24 * PP, S), fp8)
    din("wk", (24 * PP, D), fp8)
    din("pos", (D, S), f32)
    for l in range(L):
        din(f"eqk{l}", (D, 2 * D), fp8)
        din(f"evw{l}", (D, D), bf16)
        din(f"eqb{l}", (PP, 6), f32)
        din(f"eproj{l}", (D, D), bf16)
        din(f"epb{l}", (PP, 6), f32)
        din(f"em1{l}", (D, MLP), fp8)
        din(f"em1b{l}", (PP, 24), f32)
        din(f"em2{l}", (MLP, D), bf16)
        din(f"em2b{l}", (PP, 6), f32)
    din("e2dw", (D, DD), bf16)
    din("e2db", (PP, 4), f32)
    din("dq", (DD, P), f32)
    for l in range(LD):
        din(f"dwq{l}", (DD, DD), bf16)
        din(f"dqb{l}", (PP, 4), f32)
        din(f"dwk{l}", (DD, DD), bf16)
        din(f"dwv{l}", (DD, DD), bf16)
        din(f"dwo{l}", (DD, DD), bf16)
        din(f"dob{l}", (PP, 4), f32)
        din(f"dm1{l}", (DD, MLPD), bf16)
        din(f"dm1b{l}", (PP, 16), f32)
        din(f"dm2{l}", (MLPD, DD), bf16)
        din(f"dm2b{l}", (PP, 4), f32)
    din("hw", (DD, OUT), bf16)
    din("hbb", (PP, OUT), f32)
    out_ext = nc.declare_dram_parameter("out", [P, OUT], f32, isOutput=True)
    dbg_ext = {name: nc.declare_dram_parameter(name, [PP, 6, S], f32, isOutput=True)
               for name in dbg}

    with tile.TileContext(nc) as tc:
        with (
            tc.tile_pool(name="consts", bufs=1) as consts,
            tc.tile_pool(name="wpool", bufs=4) as wpool,
            tc.tile_pool(name="bias", bufs=6) as biasp,
            tc.tile_pool(name="act", bufs=1) as act,
            tc.tile_pool(name="tmp", bufs=3) as tmp,
            tc.tile_pool(name="hsqp", bufs=1) as hsqp,
            tc.tile_pool(name="bigp", bufs=1) as bigp,
            tc.tile_pool(name="attn", bufs=10) as attnp,
            tc.tile_pool(name="rrsp", bufs=2) as rrsp,
            tc.tile_pool(name="small", bufs=2) as small,
            tc.tile_pool(name="pp_mm", bufs=2, space="PSUM") as pp_mm,
            tc.tile_pool(name="pp_sc", bufs=2, space="PSUM") as pp_sc,
            tc.tile_pool(name="pp_pv", bufs=2, space="PSUM") as pp_pv,
            tc.tile_pool(name="pp_st", bufs=1, space="PSUM") as pp_st,
            tc.tile_pool(name="pp_bc", bufs=1, space="PSUM") as pp_bc,
        ):
            ones_f32 = consts.tile([PP, 1], f32)
            nc.vector.memset(ones_f32, 1.0)
            ones_row = consts.tile([1, PP], f32)
            nc.vector.memset(ones_row, 1.0)
            ones_bf16 = consts.tile([PP, 1], bf16)
            nc.vector.memset(ones_bf16, 1.0)
            ones_row_bf = consts.tile([1, PP], bf16)
            nc.vector.memset(ones_row_bf, 1.0)
            ones_full = consts.tile([PP, PP], bf16)
            nc.vector.memset(ones_full, 1.0)
            eps_t = consts.tile([1, 1], f32)
            nc.vector.memset(eps_t, EPS)
            dummy_in = consts.tile([1, 1], f32)
            nc.vector.memset(dummy_in, 0.5)

            def load_w(name, KO, M, dtype=bf16, tag="w"):
                t = wpool.tile([PP, KO, M], dtype, tag=tag)
                nc.sync.dma_start(
                    out=t, in_=dp[name][:].rearrange("(ko p) m -> p ko m", p=PP))
                return t

            W_SLOT = 6144  # elems per partition in a weight slot

            def load_b(name, MO):
                t = biasp.tile([PP, MO], f32, tag="b")
                nc.sync.dma_start(out=t, in_=dp[name][:])
                return t

            def prewarm(func):
                d = small.tile([1, 1], f32, tag="dum", name="dum")
                nc.scalar.activation(d, dummy_in, func)

            # dense matmul with chunked weight streaming from DRAM.
            # out feature-major; rhs (128, KO, N); evac(m, psum)
            def dense(wname, KO, MO, rhs_sb, N, evac, use_fp8=False):
                M = MO * PP
                mch_cols = max(PP, (W_SLOT // KO) // PP * PP)
                wap = dp[wname][:].rearrange("(ko p) m -> p ko m", p=PP)
                wdt = fp8 if use_fp8 else bf16
                for c0 in range(0, M, mch_cols):
                    mch = min(mch_cols, M - c0)
                    wt = wpool.tile([PP, KO, mch], wdt, tag="w")
                    nc.sync.dma_start(out=wt, in_=wap[:, :, c0:c0 + mch])
                    for mi in range(mch // PP):
                        m = c0 // PP + mi
                        ps = pp_mm.tile([PP, 512], f32, tag="mm")
                        if use_fp8:
                            for k in range(0, KO, 2):
                                nc.tensor.matmul(
                                    ps[:, :N],
                                    lhsT=wt[:, k:k + 2, mi * PP:(mi + 1) * PP],
                                    rhs=rhs_sb[:, k:k + 2, :],
                                    start=(k == 0), stop=(k == KO - 2),
                                    perf_mode=DR)
                        else:
                            for k in range(KO):
                                nc.tensor.matmul(
                                    ps[:, :N],
                                    lhsT=wt[:, k, mi * PP:(mi + 1) * PP],
                                    rhs=rhs_sb[:, k, :],
                                    start=(k == 0), stop=(k == KO - 1))
                        evac(m, ps[:, :N])

            # LN stats on feature-major h (128, KO, Wd) f32.
            # Fills hb (bf16 copy).  Returns (inv_bf, inv64_bf) [1,Wd] rows
            # (inv64 only if want64).  Mean-sub is folded into centered
            # weights; only 1/std survives, applied at the next evac.
            def ln_stats(h_sb, hb, KO, Wd, want1=True, want64=False):
                Dm = KO * PP
                hsq = hsqp.tile([PP, KO, Wd], bf16, tag="hsq")
                st = pp_st.tile([33, Wd], f32, tag="st")
                for k in range(KO):
                    nc.vector.tensor_copy(hb[:, k, :], h_sb[:, k, :])
                    nc.tensor.matmul(st[0:1, :], lhsT=ones_bf16, rhs=hb[:, k, :],
                                     start=(k == 0), stop=(k == KO - 1))
                    nc.scalar.activation(hsq[:, k, :], h_sb[:, k, :], AF.Square)
                    nc.tensor.matmul(st[32:33, :], lhsT=ones_bf16, rhs=hsq[:, k, :],
                                     start=(k == 0), stop=(k == KO - 1))
                mean = small.tile([1, Wd], f32, tag="s1")
                nc.vector.tensor_scalar_mul(mean, st[0:1, :], 1.0 / Dm)
                var = small.tile([1, Wd], f32, tag="s2")
                nc.vector.tensor_scalar_mul(var, st[32:33, :], 1.0 / Dm)
                msq = small.tile([1, Wd], f32, tag="s3")
                nc.vector.tensor_mul(msq, mean, mean)
                nc.vector.tensor_sub(var, var, msq)
                # ln(var+eps); rsqrt = exp(-0.5*ln): stays in exp/ln ACT set
                nc.scalar.activation(var, var, AF.Ln, bias=eps_t)
                inv_bf = inv64_bf = None
                if want1:
                    inv_bf = small.tile([1, Wd], bf16, tag="s6")
                    nc.scalar.activation(inv_bf, var, AF.Exp, scale=-0.5)
                if want64:
                    ln64 = consts_ln64[0]
                    inv64_bf = small.tile([1, Wd], bf16, tag="s7")
                    nc.scalar.activation(inv64_bf, var, AF.Exp, scale=-0.5,
                                         bias=ln64)
                return inv_bf, inv64_bf

            consts_ln64 = [consts.tile([1, 1], f32, name="ln64")]
            nc.vector.memset(consts_ln64[0], LN_W8)

            # broadcast a [1,Wd] bf16 row to a [128,Wd] bf16 SBUF tile
            def bcast_row(row_bf, Wd, name):
                bc = pp_bc.tile([PP, 512], f32, tag="bc", name=f"bc_{name}")
                nc.tensor.matmul(bc[:, :Wd], lhsT=ones_row_bf, rhs=row_bf,
                                 start=True, stop=True)
                sb = rrsp.tile([PP, 512], bf16, tag="rrs", name=f"sb_{name}")
                nc.vector.tensor_copy(sb[:, :Wd], bc[:, :Wd])
                return sb

            # inv row -> per-partition column layout [128, n_jb] (for
            # token-major evacs); returns SBUF f32 [128, n_jb]
            def inv_cols(inv_bf, n_jb, name):
                icp = pp_st.tile([PP, 4], f32, tag="st", name=f"icp_{name}")
                for jb in range(n_jb):
                    nc.tensor.matmul(
                        icp[:, jb:jb + 1],
                        lhsT=inv_bf[:, jb * PP:(jb + 1) * PP],
                        rhs=ones_bf16[0:1, 0:1],
                        start=True, stop=True)
                ics = small.tile([PP, 4], f32, tag="ics", name=f"ics_{name}")
                nc.vector.tensor_copy(ics[:, :n_jb], icp[:, :n_jb])
                return ics

            # token-major V computation with interleaved ones columns.
            # vT layout per kj block: n_pairs x [sub0 64 | 1 | sub1 64 | 1]
            def v_dense(wv, hb_t, vT_sb, KO, n_jb, Dv, ics):
                chunks = []
                c0 = 0
                while c0 < Dv:
                    cw = min(512, Dv - c0)
                    chunks.append((c0, cw))
                    c0 += cw
                for jb in range(n_jb):
                    for c0, cw in chunks:
                        ps = pp_mm.tile([PP, 512], f32, tag="mm")
                        for k in range(KO):
                            nc.tensor.matmul(
                                ps[:, :cw],
                                lhsT=hb_t[:, k, jb * PP:(jb + 1) * PP],
                                rhs=wv[:, k, c0:c0 + cw],
                                start=(k == 0), stop=(k == KO - 1))
                        npr = cw // PP
                        p0 = c0 // PP
                        src = ps[:, :cw].rearrange(
                            "t (pr s c) -> t pr s c", s=2, c=64)
                        dst = vT_sb[:, jb, p0 * 130:(p0 + npr) * 130].rearrange(
                            "t (pr s c) -> t pr s c", s=2, c=65)[:, :, :, 0:64]
                        if ics is None:
                            nc.vector.tensor_copy(dst, src)
                        else:
                            nc.vector.tensor_scalar_mul(dst, src,
                                                        ics[:, jb:jb + 1])

            # attention with transposed scores; rowsums from the V ones-col.
            def attention(q_sb, k_sb, vT_sb, o_sb, n_heads, n_q_tiles,
                          allowed_per_qi):
                kj_all = sorted({kj for qi in range(n_q_tiles)
                                 for kj in allowed_per_qi[qi]})
                kj_to_qi = {kj: [qi for qi in range(n_q_tiles)
                                 if kj in allowed_per_qi[qi]] for kj in kj_all}

                def qi_runs(qis):
                    runs = []
                    i = 0
                    while i < len(qis):
                        j = i
                        while j + 1 < len(qis) and qis[j + 1] == qis[j] + 1:
                            j += 1
                        runs.append(qis[i:j + 1])
                        i = j + 1
                    return runs

                W0 = n_q_tiles * PP
                # pack consecutive key blocks into <=512-col score tiles
                if n_q_tiles == 1:
                    packs = []
                    cur = []
                    cur_cols = 0
                    for kj in kj_all:
                        nc_kj = len(kj_to_qi[kj]) * PP
                        if cur and cur_cols + nc_kj > 512:
                            packs.append(cur)
                            cur, cur_cols = [], 0
                        cur.append(kj)
                        cur_cols += nc_kj
                    if cur:
                        packs.append(cur)
                else:
                    packs = [[kj] for kj in kj_all]

                npair = n_heads // 2
                for pair in range(npair):
                    pt2 = [{}, {}]
                    for pack in packs:
                        pcols = sum(len(kj_to_qi[kj]) for kj in pack) * PP
                        sc2 = []
                        for sub in range(2):
                            sc2.append(pp_sc.tile([PP, 512], f32, tag="sc",
                                                  name=f"sc{sub}"))
                        base = 0
                        for kj in pack:
                            qis = kj_to_qi[kj]
                            for run in qi_runs(qis):
                                col = base + qis.index(run[0]) * PP
                                for sub in range(2):
                                    b0 = 64 * sub
                                    nc.tensor.matmul(
                                        sc2[sub][:, col:col + len(run) * PP],
                                        lhsT=k_sb[b0:b0 + 64, pair,
                                                  kj * PP:(kj + 1) * PP],
                                        rhs=q_sb[b0:b0 + 64, pair,
                                                 run[0] * PP:
                                                 (run[-1] + 1) * PP],
                                        start=True, stop=True)
                            base += len(qis) * PP
                        for sub in range(2):
                            pt = attnp.tile([PP, 512], bf16, tag="p")
                            nc.scalar.activation(pt[:, :pcols],
                                                 sc2[sub][:, :pcols], AF.Exp)
                            base = 0
                            for kj in pack:
                                qis = kj_to_qi[kj]
                                pt2[sub][kj] = (pt, {qi: base + i * PP
                                                     for i, qi
                                                     in enumerate(qis)})
                                base += len(qis) * PP
                    # PV with ones-column: out rows 0-63 = o, row 64 = rowsum.
                    # Both subs at base partition 0 (65 rows incl ones);
                    # sub1 is shifted to o_sb rows 64-127 at evac (32-aligned
                    # partition shifts are legal).
                    pvs = [pp_pv.tile([65, 512], f32, tag="pv",
                                      name=f"pv{sub}") for sub in range(2)]
                    for kj in kj_all:
                        qis = kj_to_qi[kj]
                        i = 0
                        while i < len(qis):
                            qi0 = qis[i]
                            st0 = (kj == allowed_per_qi[qi0][0])
                            sp0 = (kj == allowed_per_qi[qi0][-1])
                            j = i
                            while (j + 1 < len(qis)
                                   and qis[j + 1] == qis[j] + 1
                                   and (kj == allowed_per_qi[
                                       qis[j + 1]][0]) == st0
                                   and (kj == allowed_per_qi[
                                       qis[j + 1]][-1]) == sp0):
                                j += 1
                            run = qis[i:j + 1]
                            for s2 in range(2):
                                pt, cols = pt2[s2][kj]
                                nc.tensor.matmul(
                                    pvs[s2][0:65,
                                            run[0] * PP:(run[-1] + 1) * PP],
                                    lhsT=vT_sb[:, kj,
                                               pair * 130 + s2 * 65:
                                               pair * 130 + s2 * 65 + 65],
                                    rhs=pt[:, cols[run[0]]:
                                           cols[run[0]] + len(run) * PP],
                                    start=st0, stop=sp0)
                            i = j + 1
                    # rowsum reciprocals: ln shifts row 64 -> partitions 0/32,
                    # one exp covers both subs
                    lnt = small.tile([33, 512], f32, tag="lnt")
                    nc.scalar.activation(lnt[0:1, :W0], pvs[0][64:65, :W0],
                                         AF.Ln)
                    nc.scalar.activation(lnt[32:33, :W0], pvs[1][64:65, :W0],
                                         AF.Ln)
                    rrt = small.tile([33, 512], bf16, tag="rrt")
                    nc.scalar.activation(rrt[:, :W0], lnt[:, :W0],
                                         AF.Exp, scale=-1.0)
                    # evac unnormalized PV
                    nc.vector.tensor_copy(o_sb[0:64, pair, :W0],
                                          pvs[0][0:64, :W0])
                    nc.vector.tensor_copy(o_sb[64:128, pair, :W0],
                                          pvs[1][0:64, :W0])
                    # broadcast 1/rowsum and normalize in place
                    rrb = pp_bc.tile([PP, 512], f32, tag="bc", name="rrb")
                    nc.tensor.matmul(rrb[0:64, :W0],
                                     lhsT=ones_row_bf[:, :64],
                                     rhs=rrt[0:1, :W0], start=True, stop=True)
                    nc.tensor.matmul(rrb[64:128, :W0],
                                     lhsT=ones_full[32:33, :64],
                                     rhs=rrt[32:33, :W0], start=True, stop=True)
                    rrs = rrsp.tile([PP, 512], bf16, tag="rrs", name="rrs")
                    nc.vector.tensor_copy(rrs[:, :W0], rrb[:, :W0])
                    nc.vector.tensor_tensor(
                        o_sb[:, pair, :W0], o_sb[:, pair, :W0],
                        rrs[:, :W0], OP.mult)

            # ---------------- persistent tiles ----------------
            xf_sb = bigp.tile([PP, 24, S], fp8, tag="big", name="xf_sb")
            nc.sync.dma_start(
                out=xf_sb, in_=dp["xfT"][:].rearrange("(ko p) m -> p ko m", p=PP))
            pos_sb = act.tile([PP, 6, S], f32)
            nc.sync.dma_start(
                out=pos_sb, in_=dp["pos"][:].rearrange("(ko p) m -> p ko m", p=PP))
            h_sb = act.tile([PP, 6, S], f32)
            hb_sb = act.tile([PP, 6, S], bf16)
            h8_sb = act.tile([PP, 6, S], fp8)
            q_sb = act.tile([PP, 6, S], bf16)
            k_sb = act.tile([PP, 6, S], bf16)
            vT_sb = act.tile([PP, 4, 6 * 130], bf16)
            o_sb = act.tile([PP, 6, S], bf16)
            # ones columns in vT (positions static across layers)
            for pr in range(6):
                for sub in range(2):
                    nc.vector.memset(
                        vT_sb[:, :, pr * 130 + sub * 65 + 64:
                              pr * 130 + sub * 65 + 65], 1.0)

            # ---------------- patch embedding (fp8 DR) ----------------
            def embed_evac(m, ps):
                t = tmp.tile([PP, S], f32, tag="ev")
                nc.vector.tensor_scalar_mul(t, ps, INV_W8)
                nc.vector.tensor_tensor(h_sb[:, m, :], t, pos_sb[:, m, :],
                                        OP.add)
            dense("wk", 24, 6, xf_sb, S, embed_evac, use_fp8=True)

            if "dbg_h0" in dbg_ext:
                nc.sync.dma_start(out=dbg_ext["dbg_h0"][:], in_=h_sb)

            # ---------------- encoder layers ----------------
            for l in range(L):
                # LN1: stats + inv rows; mean-sub is folded into weights
                inv_bf, inv64_bf = ln_stats(h_sb, hb_sb, 6, S,
                                            want1=True, want64=True)
                inv64_sb = bcast_row(inv64_bf, S, f"i64_{l}")
                ics = inv_cols(inv_bf, 4, f"l{l}")
                for k in range(6):
                    nc.vector.tensor_copy(h8_sb[:, k, :], hb_sb[:, k, :])

                qb = load_b(f"eqb{l}", 6)

                def qkv_evac(m, ps):
                    if m < 6:      # Q: *inv/64 then +bias
                        nc.vector.tensor_tensor(q_sb[:, m, :], ps,
                                                inv64_sb, OP.mult)
                        nc.vector.tensor_scalar_add(q_sb[:, m, :], q_sb[:, m, :],
                                                    qb[:, m:m + 1])
                    else:          # K: *inv/64
                        nc.vector.tensor_tensor(k_sb[:, m - 6, :], ps,
                                                inv64_sb, OP.mult)
                dense(f"eqk{l}", 6, 12, h8_sb, S, qkv_evac, use_fp8=True)

                # V (bf16, token-major, *inv at evac via per-partition cols)
                wv = load_w(f"evw{l}", 6, D)
                v_dense(wv, hb_sb, vT_sb, 6, 4, D, ics)

                attention(q_sb, k_sb, vT_sb, o_sb, NH, 4, allowed[l])

                pb = load_b(f"epb{l}", 6)

                def proj_evac(m, ps):
                    t = tmp.tile([PP, S], f32, tag="ev")
                    nc.vector.tensor_scalar_add(t, ps, pb[:, m:m + 1])
                    nc.gpsimd.tensor_tensor(h_sb[:, m, :], h_sb[:, m, :], t,
                                            OP.add)
                dense(f"eproj{l}", 6, 6, o_sb, S, proj_evac)

                # LN2 + MLP
                _, inv64_2bf = ln_stats(h_sb, hb_sb, 6, S,
                                        want1=False, want64=True)
                inv64_2sb = bcast_row(inv64_2bf, S, f"i642_{l}")
                for k in range(6):
                    nc.vector.tensor_copy(h8_sb[:, k, :], hb_sb[:, k, :])
                prewarm(AF.Gelu)

                g_sb = bigp.tile([PP, 24, S], bf16, tag="big", name="g_sb")
                m1b = load_b(f"em1b{l}", 24)

                def gelu_evac(m, ps):
                    t = tmp.tile([PP, S], f32, tag="ev")
                    nc.vector.tensor_tensor(t, ps, inv64_2sb, OP.mult)
                    nc.scalar.activation(g_sb[:, m, :], t, AF.Gelu,
                                         bias=m1b[:, m:m + 1])
                dense(f"em1{l}", 6, 24, h8_sb, S, gelu_evac, use_fp8=True)
                prewarm(AF.Ln)

                m2b = load_b(f"em2b{l}", 6)

                def mlp2_evac(m, ps):
                    t = tmp.tile([PP, S], f32, tag="ev")
                    nc.vector.tensor_scalar_add(t, ps, m2b[:, m:m + 1])
                    nc.gpsimd.tensor_tensor(h_sb[:, m, :], h_sb[:, m, :], t,
                                            OP.add)
                dense(f"em2{l}", 24, 6, g_sb, S, mlp2_evac)

                if f"dbg_he{l}" in dbg_ext:
                    nc.sync.dma_start(out=dbg_ext[f"dbg_he{l}"][:], in_=h_sb)

            # ---------------- encoder -> decoder ----------------
            for k in range(6):
                nc.vector.tensor_copy(hb_sb[:, k, :], h_sb[:, k, :])
            e2db = load_b("e2db", 4)
            memT_sb = act.tile([PP, 4, S], bf16)   # feature-major mem

            def e2d_evac(m, ps):
                nc.vector.tensor_scalar_add(memT_sb[:, m, :], ps,
                                            e2db[:, m:m + 1])
            dense("e2dw", 6, 4, hb_sb, S, e2d_evac)

            # ---------------- decoder ----------------
            qd_sb = act.tile([PP, 4, P], f32)      # decoder residual stream
            nc.sync.dma_start(
                out=qd_sb, in_=dp["dq"][:].rearrange("(ko p) m -> p ko m", p=PP))

            qdb_sb = act.tile([PP, 4, P], bf16)
            Qd_sb = act.tile([PP, 4, P], bf16)
            Kd_sb = act.tile([PP, 4, S], bf16)
            vTd_sb = act.tile([PP, 4, 4 * 130], bf16)
            od_sb = act.tile([PP, 4, P], bf16)
            gd_sb = act.tile([PP, 16, P], bf16)
            for pr in range(4):
                for sub in range(2):
                    nc.vector.memset(
                        vTd_sb[:, :, pr * 130 + sub * 65 + 64:
                               pr * 130 + sub * 65 + 65], 1.0)

            for l in range(LD):

                def kd_evac(m, ps):
                    nc.vector.tensor_copy(Kd_sb[:, m, :], ps)
                dense(f"dwk{l}", 4, 4, memT_sb, S, kd_evac)

                wvd = load_w(f"dwv{l}", 4, DD)
                v_dense(wvd, memT_sb, vTd_sb, 4, 4, DD, None)

                inv_dbf, _ = ln_stats(qd_sb, qdb_sb, 4, P, want1=True)
                inv_dsb = bcast_row(inv_dbf, P, f"id_{l}")
                qbd = load_b(f"dqb{l}", 4)

                def qd_evac(m, ps):
                    nc.vector.tensor_tensor(Qd_sb[:, m, :], ps, inv_dsb[:, :P],
                                            OP.mult)
                    nc.vector.tensor_scalar_add(Qd_sb[:, m, :], Qd_sb[:, m, :],
                                                qbd[:, m:m + 1])
                dense(f"dwq{l}", 4, 4, qdb_sb, P, qd_evac)

                attention(Qd_sb, Kd_sb, vTd_sb, od_sb, NHD, 1,
                          [(0, 1, 2, 3)])

                obd = load_b(f"dob{l}", 4)

                def od_evac(m, ps):
                    t = tmp.tile([PP, S], f32, tag="ev")
                    nc.vector.tensor_scalar_add(t[:, :P], ps, obd[:, m:m + 1])
                    nc.gpsimd.tensor_tensor(qd_sb[:, m, :], qd_sb[:, m, :],
                                            t[:, :P], OP.add)
                dense(f"dwo{l}", 4, 4, od_sb, P, od_evac)

                inv_d2bf, _ = ln_stats(qd_sb, qdb_sb, 4, P, want1=True)
                inv_d2sb = bcast_row(inv_d2bf, P, f"id2_{l}")
                prewarm(AF.Gelu)
                m1bd = load_b(f"dm1b{l}", 16)

                def gelud_evac(m, ps):
                    t = tmp.tile([PP, S], f32, tag="ev")
                    nc.vector.tensor_tensor(t[:, :P], ps, inv_d2sb[:, :P],
                                            OP.mult)
                    nc.scalar.activation(gd_sb[:, m, :], t[:, :P], AF.Gelu,
                                         bias=m1bd[:, m:m + 1])
                dense(f"dm1{l}", 4, 16, qdb_sb, P, gelud_evac)
                prewarm(AF.Ln)

                m2bd = load_b(f"dm2b{l}", 4)

                def mlp2d_evac(m, ps):
                    t = tmp.tile([PP, S], f32, tag="ev")
                    nc.vector.tensor_scalar_add(t[:, :P], ps, m2bd[:, m:m + 1])
                    nc.gpsimd.tensor_tensor(qd_sb[:, m, :], qd_sb[:, m, :],
                                            t[:, :P], OP.add)
                dense(f"dm2{l}", 16, 4, gd_sb, P, mlp2d_evac)

            # ---------------- head ----------------
            inv_hbf, _ = ln_stats(qd_sb, qdb_sb, 4, P, want1=True)
            ics_h = inv_cols(inv_hbf, 1, "hd")
            wh = load_w("hw", 4, OUT)
            hbb_sb = act.tile([PP, OUT], f32)
            nc.sync.dma_start(out=hbb_sb, in_=dp["hbb"][:])
            out_sb = act.tile([P, OUT], f32)
            for nchunk in range(2):
                ncs = slice(nchunk * 384, (nchunk + 1) * 384)
                ps = pp_mm.tile([PP, 512], f32, tag="mm")
                for k in range(4):
                    nc.tensor.matmul(ps[:, :384],
                                     lhsT=qdb_sb[:, k, :],
                                     rhs=wh[:, k, ncs],
                                     start=(k == 0), stop=(k == 3))
                t = tmp.tile([PP, S], f32, tag="ev")
                nc.vector.tensor_scalar_mul(t[:, :384], ps[:, :384],
                                            ics_h[:, 0:1])
                nc.vector.tensor_tensor(out_sb[:, ncs], t[:, :384],
                                        hbb_sb[:, ncs], OP.add)
            nc.sync.dma_start(out=out_ext[:], in_=out_sb)

    return nc


# ------------------------------------------------------------------
# entry point
# ------------------------------------------------------------------

def kernel(dbg=(), _trace=False, _tmpdir=None, _full=False, **inputs):
    from concourse.bass_utils import run_bass_kernel_spmd

    w, xfT_f8, allowed = _prep(inputs)
    nc = _build(allowed, dbg=dbg)
    in_maps = []
    for b in range(B):
        m = dict(w)
        m["xfT"] = xfT_f8[b]
        in_maps.append(m)
    res = run_bass_kernel_spmd(nc, in_maps, core_ids=list(range(8)),
                               trace=_trace, tmpdir=_tmpdir)
    out = np.stack([np.asarray(res.results[i]["out"]) for i in range(B)])
    if dbg or _full:
        dbgs = {name: np.stack([np.asarray(res.results[i][name])
                                for i in range(B)]) for name in dbg}
        return out.astype(np.float32), dbgs, res
    return out.astype(np.float32)


# revision 12
# speedup vs baseline: 1.3167x; 1.0974x over previous
"""Trainium2 Bass kernel for nn_ARPredVideoVanilla (8-core data-parallel).

Strategy: pure data parallelism over batch (B=8 -> 1 batch element per core,
no collectives).  Activations live feature-major in SBUF.  Key optimizations
over the bf16 baseline:

- fp8(e4m3) DoubleRow matmuls (2 contraction rows per PE cell, ~1.44x) for the
  quantization-tolerant GEMMs: patch embed, Q/K projection, MLP fc1.  Weights
  are scaled x64 into the e4m3 normal range; the 1/64 is folded into the
  PSUM-evacuation ops.
- LayerNorm mean-subtraction is folded into the next matmul by column-centering
  its weights on the host (exact identity); the 1/std multiply is deferred to
  the PSUM evacuation.  The LN serial chain no longer blocks the PE.
- Softmax row sums come for free from an appended ones-column in V (the PV
  matmul's 65th output row); reciprocals are computed batched (ln+exp over all
  head rowsums at once, on parallel partitions).
- ACT table-set switches (natural_log_exp <-> gelu) are prewarmed with dummy
  activations so the ~2.7us loads overlap matmul phases.
"""

import sys

sys.path.insert(0, "/opt/trn_rl_repo")

import numpy as np
import ml_dtypes

BF16 = ml_dtypes.bfloat16
F8 = ml_dtypes.float8_e4m3
W8SCALE = 64.0

# ---- model dims (hardcoded from the problem spec) ----
B, T, V = 8, 4, 3
C = V * T                      # 12
H, W, PH, PW = 128, 256, 16, 16
HP, WP = H // PH, W // PW      # 8, 16
P = HP * WP                    # 128 patches/frame
S = T * P                      # 512
D, NH, HD, L = 768, 12, 64, 8
DD, NHD, HDD, LD = 512, 8, 64, 4
MLP, MLPD = 3072, 2048
OUT = PH * PW * V              # 768
MASK_RATIO = 0.8
EPS = 1e-5
PP = 128  # partitions


# ------------------------------------------------------------------
# host-side preparation: fold biases/scales, center, transpose, cast
# ------------------------------------------------------------------

def _prep(inputs):
    f32 = np.float32
    g = {k: np.asarray(v, dtype=f32) for k, v in inputs.items()}

    w = {}

    def bf(a):
        return np.ascontiguousarray(a.astype(BF16))

    def f8w(a):  # fp8 weight with x64 scaling
        return np.ascontiguousarray((a * W8SCALE).astype(F8))

    def cc(a):  # column-center (folds LN mean subtraction)
        return a - a.mean(axis=0, keepdims=True)

    def pcol(bias):  # (M,) -> (128, M//128) per-partition layout, tile-major
        M = bias.shape[0]
        return np.ascontiguousarray(bias.reshape(M // PP, PP).T.astype(f32))

    # patch data, per core: x[b] (T,C,H,W) -> xfT (C*PH*PW, T*P) in fp8
    x = g["x"]  # (B,T,C,H,W)
    xf = x.reshape(B, T, C, HP, PH, WP, PW).transpose(0, 1, 3, 5, 2, 4, 6)
    xf = xf.reshape(B, T * P, C * PH * PW)          # (B, 512, 3072)
    xfT = np.ascontiguousarray(np.swapaxes(xf, 1, 2))  # (B, 3072, 512)
    xfT_f8 = [np.ascontiguousarray(xfT[b].astype(F8)) for b in range(B)]

    # conv: wk (3072, 768) fp8 x64; pos_eff (768, 512) f32 with conv_b folded
    wk = g["conv_w"].reshape(D, C * PH * PW).T      # (3072, 768)
    w["wk"] = f8w(wk)
    pos = g["pos_emb"][0].T + g["conv_b"][:, None]  # (768, 512)
    w["pos"] = bf(pos)

    scale = HD ** -0.5
    for l in range(L):
        s1, b1 = g["enc_ln1_s"][l], g["enc_ln1_b"][l]
        Wqkv = g["enc_qkv_w"][l]                    # (768, 2304)
        Ws = s1[:, None] * Wqkv
        cb = b1 @ Wqkv                              # LN bias folded through qkv
        Wc = cc(Ws)                                 # center (mean-sub fold)
        Wqk = Wc[:, :2 * D].copy()
        Wqk[:, :D] *= scale
        w[f"eqk{l}"] = f8w(Wqk)                     # fp8 x64
        w[f"evw{l}"] = bf(Wc[:, 2 * D:] * W8SCALE)  # V bf16 (x64 exact)
        w[f"eqb{l}"] = pcol(cb[:D] * scale)         # q bias (per-partition)
        # k bias dropped (softmax row-invariant); v bias folded into proj bias
        w[f"eproj{l}"] = bf(g["enc_proj_w"][l])
        pb = g["enc_proj_b"][l] + cb[2 * D:] @ g["enc_proj_w"][l]
        w[f"epb{l}"] = pcol(pb)
        s2, b2 = g["enc_ln2_s"][l], g["enc_ln2_b"][l]
        W1 = g["enc_mlp_w1"][l]
        w[f"em1{l}"] = f8w(cc(s2[:, None] * W1))    # fp8 x64 centered
        w[f"em1b{l}"] = pcol(b2 @ W1 + g["enc_mlp_b1"][l])
        w[f"em2{l}"] = bf(g["enc_mlp_w2"][l])
        w[f"em2b{l}"] = pcol(g["enc_mlp_b2"][l])

    w["e2dw"] = bf(g["e2d_w"])                      # (768, 512)
    w["e2db"] = pcol(g["e2d_b"])
    w["dq"] = np.ascontiguousarray(g["dec_query"][0].T.astype(f32))  # (512,128)

    dscale = HDD ** -0.5
    for l in range(LD):
        s1, b1 = g["dec_ln1_s"][l], g["dec_ln1_b"][l]
        Wq = g["dec_qkv_w"][l, 0]
        w[f"dwq{l}"] = bf(cc(s1[:, None] * Wq) * dscale)
        w[f"dqb{l}"] = pcol((b1 @ Wq + g["dec_qkv_b"][l, 0]) * dscale)
        w[f"dwk{l}"] = bf(g["dec_qkv_w"][l, 1])     # k bias dropped
        w[f"dwv{l}"] = bf(g["dec_qkv_w"][l, 2])
        w[f"dwo{l}"] = bf(g["dec_out_w"][l])
        ob = g["dec_out_b"][l] + g["dec_qkv_b"][l, 2] @ g["dec_out_w"][l]
        w[f"dob{l}"] = pcol(ob)
        s2, b2 = g["dec_ln2_s"][l], g["dec_ln2_b"][l]
        W1 = g["dec_mlp_w1"][l]
        w[f"dm1{l}"] = bf(cc(s2[:, None] * W1))
        w[f"dm1b{l}"] = pcol(b2 @ W1 + g["dec_mlp_b1"][l])
        w[f"dm2{l}"] = bf(g["dec_mlp_w2"][l])
        w[f"dm2b{l}"] = pcol(g["dec_mlp_b2"][l])

    sh, bh = g["head_ln_s"], g["head_ln_b"]
    w["hw"] = bf(cc(sh[:, None] * g["head_w"]))     # (512, 768) centered
    hb = bh @ g["head_w"] + g["head_b"]             # (768,) per-FREE bias
    w["hbb"] = np.ascontiguousarray(
        np.broadcast_to(hb[None, :], (PP, OUT)).astype(f32))

    # block mask: allowed[l][qi] = tuple of allowed key-frame blocks
    mr = g["mask_rand"]                             # (L, T, T)
    allowed = []
    for l in range(L):
        per_q = []
        for i in range(T):
            ks = [j for j in range(T)
                  if j <= i or not (mr[l, i, j] < MASK_RATIO)]
            per_q.append(tuple(ks))
        allowed.append(per_q)

    return w, xfT_f8, allowed


# ------------------------------------------------------------------
# Tile tail-drain patch: this walrus build rejects >1 sync wait per
# instruction at the kernel-tail drain; split the waits across NOPs.
# ------------------------------------------------------------------

def _patch_tile():
    import concourse.tile as tile
    from concourse.vector_clock import ScopedClock, VectorClock

    if getattr(tile.TileContext, "_drain_patched", False):
        return

    def _drain_and_barrier_chunked(self, tick_clock, wait_clock):
        g = list(tick_clock.global_clock)
        procs = [i for i, v in enumerate(g) if v > 0]
        for p in procs:
            sub = [0] * len(g)
            sub[p] = g[p]
            nop_inst = self.nc.sync.nop(nofuse=True)
            wait_clock.add_sem_waits(
                nop_inst.ins, ScopedClock({None: VectorClock(sub)}))
        self.nc.sync.drain()
        self.nc.all_engine_barrier()
        assert self.sems is not None
        popped = self.nc._tile_sem_poison_stack.pop()
        assert popped is self._sem_poison
        self.nc.clear_and_free_semaphores(list(self.sems.allocated().values()))
        self.nc.all_engine_barrier()

    tile.TileContext._drain_and_barrier = _drain_and_barrier_chunked

    # This walrus build also rejects >1 sync wait on regular engine
    # instructions (Matmult etc.).  Hoist excess waits onto same-engine
    # NOPs inserted immediately before the instruction.
    from concourse import mybir as _mybir

    _orig_lower = tile.TileContext._lower_ordered_insts

    def _split_waits_and_lower(self, ordered):
        nctr = [0]
        for bb_name, insts in ordered.items():
            new_list = []
            for inst in insts:
                si = getattr(inst, "sync_info", None)
                waits = list(si.on_wait) if si is not None else []
                if len(waits) > 1:
                    imm = [w for w in waits if w.wait_reg is None]
                    reg = [w for w in waits if w.wait_reg is not None]
                    keep = imm[:1] + reg  # keep one imm (plus any reg waits)
                    excess = imm[1:]
                    for w in excess:
                        nctr[0] += 1
                        nop = _mybir.InstNoOp(
                            name=f"{inst.name}-wsplit{nctr[0]}", ins=[], outs=[])
                        nop.engine = inst.engine
                        nop.sync_info = _mybir.SyncInfo(
                            on_wait=[w], on_update=[])
                        self.nc.register_instruction(nop, overwrite=True)
                        new_list.append(nop)
                    inst.sync_info = _mybir.SyncInfo(
                        on_wait=keep, on_update=list(si.on_update))
                new_list.append(inst)
            insts[:] = new_list
        return _orig_lower(self, ordered)

    tile.TileContext._lower_ordered_insts = _split_waits_and_lower
    tile.TileContext._drain_patched = True


# ------------------------------------------------------------------
# graph builder
# ------------------------------------------------------------------

def _build(allowed, dbg=()):
    import concourse.bass as bass
    import concourse.tile as tile
    from concourse import mybir

    _patch_tile()
    f32 = mybir.dt.float32
    bf16 = mybir.dt.bfloat16
    fp8 = mybir.dt.float8e4
    AF = mybir.ActivationFunctionType
    OP = mybir.AluOpType
    DR = mybir.MatmulPerfMode.DoubleRow
    INV_W8 = 1.0 / W8SCALE
    LN_W8 = float(np.log(1.0 / W8SCALE))

    nc = bass.Bass()

    # ---- DRAM parameters ----
    dp = {}

    def din(name, shape, dtype):
        dp[name] = nc.declare_dram_parameter(name, list(shape), dtype, isOutput=False)
        return dp[name]

    din("xfT", (24 * PP, S), fp8)
    din("wk", (24 * PP, D), fp8)
    din("pos", (D, S), bf16)
    for l in range(L):
        din(f"eqk{l}", (D, 2 * D), fp8)
        din(f"evw{l}", (D, D), bf16)
        din(f"eqb{l}", (PP, 6), f32)
        din(f"eproj{l}", (D, D), bf16)
        din(f"epb{l}", (PP, 6), f32)
        din(f"em1{l}", (D, MLP), fp8)
        din(f"em1b{l}", (PP, 24), f32)
        din(f"em2{l}", (MLP, D), bf16)
        din(f"em2b{l}", (PP, 6), f32)
    din("e2dw", (D, DD), bf16)
    din("e2db", (PP, 4), f32)
    din("dq", (DD, P), f32)
    for l in range(LD):
        din(f"dwq{l}", (DD, DD), bf16)
        din(f"dqb{l}", (PP, 4), f32)
        din(f"dwk{l}", (DD, DD), bf16)
        din(f"dwv{l}", (DD, DD), bf16)
        din(f"dwo{l}", (DD, DD), bf16)
        din(f"dob{l}", (PP, 4), f32)
        din(f"dm1{l}", (DD, MLPD), bf16)
        din(f"dm1b{l}", (PP, 16), f32)
        din(f"dm2{l}", (MLPD, DD), bf16)
        din(f"dm2b{l}", (PP, 4), f32)
    din("hw", (DD, OUT), bf16)
    din("hbb", (PP, OUT), f32)
    out_ext = nc.declare_dram_parameter("out", [P, OUT], f32, isOutput=True)
    dbg_ext = {name: nc.declare_dram_parameter(name, [PP, 6, S], f32, isOutput=True)
               for name in dbg}

    with tile.TileContext(nc) as tc:
        with (
            tc.tile_pool(name="consts", bufs=1) as consts,
            tc.tile_pool(name="wpool", bufs=3) as wpool,
            tc.tile_pool(name="bias", bufs=6) as biasp,
            tc.tile_pool(name="act", bufs=1) as act,
            tc.tile_pool(name="tmp", bufs=6) as tmp,
            tc.tile_pool(name="hsqp", bufs=1) as hsqp,
            tc.tile_pool(name="bigp", bufs=1) as bigp,
            tc.tile_pool(name="attn", bufs=10) as attnp,
            tc.tile_pool(name="rrsp", bufs=2) as rrsp,
            tc.tile_pool(name="small", bufs=2) as small,
            tc.tile_pool(name="pp_mm", bufs=2, space="PSUM") as pp_mm,
            tc.tile_pool(name="pp_sc", bufs=2, space="PSUM") as pp_sc,
            tc.tile_pool(name="pp_pv", bufs=2, space="PSUM") as pp_pv,
            tc.tile_pool(name="pp_st", bufs=1, space="PSUM") as pp_st,
            tc.tile_pool(name="pp_bc", bufs=1, space="PSUM") as pp_bc,
        ):
            ones_f32 = consts.tile([PP, 1], f32)
            nc.vector.memset(ones_f32, 1.0)
            ones_row = consts.tile([1, PP], f32)
            nc.vector.memset(ones_row, 1.0)
            ones_bf16 = consts.tile([PP, 1], bf16)
            nc.vector.memset(ones_bf16, 1.0)
            ones_row_bf = consts.tile([1, PP], bf16)
            nc.vector.memset(ones_row_bf, 1.0)
            ones_full = consts.tile([PP, PP], bf16)
            nc.vector.memset(ones_full, 1.0)
            eps_t = consts.tile([1, 1], f32)
            nc.vector.memset(eps_t, EPS)
            dummy_in = consts.tile([1, 1], f32)
            nc.vector.memset(dummy_in, 0.5)

            def load_w(name, KO, M, dtype=bf16, tag="w"):
                t = wpool.tile([PP, KO, M], dtype, tag=tag)
                nc.sync.dma_start(
                    out=t, in_=dp[name][:].rearrange("(ko p) m -> p ko m", p=PP))
                return t

            W_SLOT = 6144  # elems per partition in a weight slot

            def load_b(name, MO):
                t = biasp.tile([PP, MO], f32, tag="b")
                nc.sync.dma_start(out=t, in_=dp[name][:])
                return t

            def prewarm(func):
                d = small.tile([1, 1], f32, tag="dum", name="dum")
                nc.scalar.activation(d, dummy_in, func)

            # dense matmul with chunked weight streaming from DRAM.
            # out feature-major; rhs (128, KO, N); evac(m, psum).
            # PSUM rotates over the mm/sc/pv pools (6 banks) so the PE can
            # run ahead of evacuation.  `defer` holds back the first few
            # evacs and emits `mid` between the matmuls and those evacs --
            # used to keep the PE queue free of the inv-broadcast matmul
            # that waits on the LN small-op chain.
            psum_rot = [0]

            def psum_tile():
                pools = ((pp_mm, "mm"), (pp_sc, "sc"), (pp_pv, "pv"))
                pool, tg = pools[psum_rot[0] % 3]
                psum_rot[0] += 1
                return pool.tile([PP, 512], f32, tag=tg, name=f"d{tg}")

            def dense(wname, KO, MO, rhs_sb, N, evac, use_fp8=False,
                      defer=0, mid=None):
                M = MO * PP
                mch_cols = max(PP, (W_SLOT // KO) // PP * PP)
                wap = dp[wname][:].rearrange("(ko p) m -> p ko m", p=PP)
                wdt = fp8 if use_fp8 else bf16
                held = []
                for c0 in range(0, M, mch_cols):
                    mch = min(mch_cols, M - c0)
                    wt = wpool.tile([PP, KO, mch], wdt, tag="w")
                    nc.sync.dma_start(out=wt, in_=wap[:, :, c0:c0 + mch])
                    for mi in range(mch // PP):
                        m = c0 // PP + mi
                        ps = psum_tile()
                        if use_fp8:
                            for k in range(0, KO, 2):
                                nc.tensor.matmul(
                                    ps[:, :N],
                                    lhsT=wt[:, k:k + 2, mi * PP:(mi + 1) * PP],
                                    rhs=rhs_sb[:, k:k + 2, :],
                                    start=(k == 0), stop=(k == KO - 2),
                                    perf_mode=DR)
                        else:
                            for k in range(KO):
                                nc.tensor.matmul(
                                    ps[:, :N],
                                    lhsT=wt[:, k, mi * PP:(mi + 1) * PP],
                                    rhs=rhs_sb[:, k, :],
                                    start=(k == 0), stop=(k == KO - 1))
                        if m < defer:
                            held.append((m, ps))
                            if m == defer - 1:
                                if mid is not None:
                                    mid()
                                for hm, hps in held:
                                    evac(hm, hps[:, :N])
                        else:
                            evac(m, ps[:, :N])
                if MO <= defer - 1:
                    if mid is not None:
                        mid()
                    for hm, hps in held:
                        evac(hm, hps[:, :N])

            # LN stats on feature-major h (128, KO, Wd) f32.
            # Fills hb (bf16 copy).  Returns (inv_bf, inv64_bf) [1,Wd] rows
            # (inv64 only if want64).  Mean-sub is folded into centered
            # weights; only 1/std survives, applied at the next evac.
            def ln_stats(h_sb, hb, KO, Wd, want64=False):
                Dm = KO * PP
                hsq = hsqp.tile([PP, KO, Wd], bf16, tag="hsq")
                st = pp_st.tile([33, Wd], f32, tag="st")
                for k in range(KO):
                    nc.vector.tensor_copy(hb[:, k, :], h_sb[:, k, :])
                    nc.tensor.matmul(st[0:1, :], lhsT=ones_bf16, rhs=hb[:, k, :],
                                     start=(k == 0), stop=(k == KO - 1))
                    nc.scalar.activation(hsq[:, k, :], h_sb[:, k, :], AF.Square)
                    nc.tensor.matmul(st[32:33, :], lhsT=ones_bf16, rhs=hsq[:, k, :],
                                     start=(k == 0), stop=(k == KO - 1))
                # var*D = st32 - st0^2/D  (Square's free affine gives /D)
                msqD = small.tile([1, Wd], f32, tag="s1")
                nc.scalar.activation(msqD, st[0:1, :], AF.Square,
                                     scale=Dm ** -0.5)
                varD = small.tile([1, Wd], f32, tag="s2")
                nc.vector.tensor_sub(varD, st[32:33, :], msqD)
                lnv = small.tile([1, Wd], f32, tag="s3")
                nc.scalar.activation(lnv, varD, AF.Ln, scale=1.0 / Dm,
                                     bias=eps_t)
                inv_bf = small.tile([1, Wd], bf16, tag="s6")
                if want64:
                    nc.scalar.activation(inv_bf, lnv, AF.Exp, scale=-0.5,
                                         bias=consts_ln64[0])
                else:
                    nc.scalar.activation(inv_bf, lnv, AF.Exp, scale=-0.5)
                return inv_bf

            consts_ln64 = [consts.tile([1, 1], f32, name="ln64")]
            nc.vector.memset(consts_ln64[0], LN_W8)

            # broadcast a [1,Wd] bf16 row to a [128,Wd] bf16 SBUF tile
            def bcast_row(row_bf, Wd, name):
                bc = pp_bc.tile([PP, 512], f32, tag="bc", name=f"bc_{name}")
                nc.tensor.matmul(bc[:, :Wd], lhsT=ones_row_bf, rhs=row_bf,
                                 start=True, stop=True)
                sb = rrsp.tile([PP, 512], bf16, tag="rrs", name=f"sb_{name}")
                nc.vector.tensor_copy(sb[:, :Wd], bc[:, :Wd])
                return sb

            # inv row -> per-partition column layout [128, n_jb] (for
            # token-major evacs); returns SBUF f32 [128, n_jb]
            def inv_cols(inv_bf, n_jb, name):
                icp = pp_st.tile([PP, 4], f32, tag="st", name=f"icp_{name}")
                for jb in range(n_jb):
                    nc.tensor.matmul(
                        icp[:, jb:jb + 1],
                        lhsT=inv_bf[:, jb * PP:(jb + 1) * PP],
                        rhs=ones_bf16[0:1, 0:1],
                        start=True, stop=True)
                ics = small.tile([PP, 4], f32, tag="ics", name=f"ics_{name}")
                nc.vector.tensor_copy(ics[:, :n_jb], icp[:, :n_jb])
                return ics

            # token-major V computation with interleaved ones columns.
            # vT layout per kj block: n_pairs x [sub0 64 | 1 | sub1 64 | 1]
            def v_dense(wv, hb_t, vT_sb, KO, n_jb, Dv, ics):
                chunks = []
                c0 = 0
                while c0 < Dv:
                    cw = min(512, Dv - c0)
                    chunks.append((c0, cw))
                    c0 += cw
                for jb in range(n_jb):
                    for c0, cw in chunks:
                        ps = psum_tile()
                        for k in range(KO):
                            nc.tensor.matmul(
                                ps[:, :cw],
                                lhsT=hb_t[:, k, jb * PP:(jb + 1) * PP],
                                rhs=wv[:, k, c0:c0 + cw],
                                start=(k == 0), stop=(k == KO - 1))
                        npr = cw // PP
                        p0 = c0 // PP
                        src = ps[:, :cw].rearrange(
                            "t (pr s c) -> t pr s c", s=2, c=64)
                        dst = vT_sb[:, jb, p0 * 130:(p0 + npr) * 130].rearrange(
                            "t (pr s c) -> t pr s c", s=2, c=65)[:, :, :, 0:64]
                        if ics is None:
                            nc.vector.tensor_copy(dst, src)
                        else:
                            nc.vector.tensor_scalar_mul(dst, src,
                                                        ics[:, jb:jb + 1])

            # attention with transposed scores; rowsums from the V ones-col.
            def attention(q_sb, k_sb, vT_sb, o_sb, n_heads, n_q_tiles,
                          allowed_per_qi):
                kj_all = sorted({kj for qi in range(n_q_tiles)
                                 for kj in allowed_per_qi[qi]})
                kj_to_qi = {kj: [qi for qi in range(n_q_tiles)
                                 if kj in allowed_per_qi[qi]] for kj in kj_all}

                def qi_runs(qis):
                    runs = []
                    i = 0
                    while i < len(qis):
                        j = i
                        while j + 1 < len(qis) and qis[j + 1] == qis[j] + 1:
                            j += 1
                        runs.append(qis[i:j + 1])
                        i = j + 1
                    return runs

                W0 = n_q_tiles * PP
                # pack consecutive key blocks into <=512-col score tiles
                if n_q_tiles == 1:
                    packs = []
                    cur = []
                    cur_cols = 0
                    for kj in kj_all:
                        nc_kj = len(kj_to_qi[kj]) * PP
                        if cur and cur_cols + nc_kj > 512:
                            packs.append(cur)
                            cur, cur_cols = [], 0
                        cur.append(kj)
                        cur_cols += nc_kj
                    if cur:
                        packs.append(cur)
                else:
                    packs = [[kj] for kj in kj_all]

                npair = n_heads // 2
                for pair in range(npair):
                    pt2 = [{}, {}]
                    for pack in packs:
                        pcols = sum(len(kj_to_qi[kj]) for kj in pack) * PP
                        sc2 = []
                        for sub in range(2):
                            sc2.append(pp_sc.tile([PP, 512], f32, tag="sc",
                                                  name=f"sc{sub}"))
                        base = 0
                        for kj in pack:
                            qis = kj_to_qi[kj]
                            for run in qi_runs(qis):
                                col = base + qis.index(run[0]) * PP
                                for sub in range(2):
                                    b0 = 64 * sub
                                    nc.tensor.matmul(
                                        sc2[sub][:, col:col + len(run) * PP],
                                        lhsT=k_sb[b0:b0 + 64, pair,
                                                  kj * PP:(kj + 1) * PP],
                                        rhs=q_sb[b0:b0 + 64, pair,
                                                 run[0] * PP:
                                                 (run[-1] + 1) * PP],
                                        start=True, stop=True)
                            base += len(qis) * PP
                        for sub in range(2):
                            pt = attnp.tile([PP, 512], bf16, tag="p")
                            nc.scalar.activation(pt[:, :pcols],
                                                 sc2[sub][:, :pcols], AF.Exp)
                            base = 0
                            for kj in pack:
                                qis = kj_to_qi[kj]
                                pt2[sub][kj] = (pt, {qi: base + i * PP
                                                     for i, qi
                                                     in enumerate(qis)})
                                base += len(qis) * PP
                    # PV with ones-column: out rows 0-63 = o, row 64 = rowsum.
                    # Both subs at base partition 0 (65 rows incl ones);
                    # sub1 is shifted to o_sb rows 64-127 at evac (32-aligned
                    # partition shifts are legal).
                    pvs = [pp_pv.tile([65, 512], f32, tag="pv",
                                      name=f"pv{sub}") for sub in range(2)]
                    for kj in kj_all:
                        qis = kj_to_qi[kj]
                        i = 0
                        while i < len(qis):
                            qi0 = qis[i]
                            st0 = (kj == allowed_per_qi[qi0][0])
                            sp0 = (kj == allowed_per_qi[qi0][-1])
                            j = i
                            while (j + 1 < len(qis)
                                   and qis[j + 1] == qis[j] + 1
                                   and (kj == allowed_per_qi[
                                       qis[j + 1]][0]) == st0
                                   and (kj == allowed_per_qi[
                                       qis[j + 1]][-1]) == sp0):
                                j += 1
                            run = qis[i:j + 1]
                            for s2 in range(2):
                                pt, cols = pt2[s2][kj]
                                nc.tensor.matmul(
                                    pvs[s2][0:65,
                                            run[0] * PP:(run[-1] + 1) * PP],
                                    lhsT=vT_sb[:, kj,
                                               pair * 130 + s2 * 65:
                                               pair * 130 + s2 * 65 + 65],
                                    rhs=pt[:, cols[run[0]]:
                                           cols[run[0]] + len(run) * PP],
                                    start=st0, stop=sp0)
                            i = j + 1
                    # rowsum reciprocals: ln shifts row 64 -> partitions 0/32,
                    # one exp covers both subs
                    lnt = small.tile([33, 512], f32, tag="lnt")
                    nc.scalar.activation(lnt[0:1, :W0], pvs[0][64:65, :W0],
                                         AF.Ln)
                    nc.scalar.activation(lnt[32:33, :W0], pvs[1][64:65, :W0],
                                         AF.Ln)
                    rrt = small.tile([33, 512], bf16, tag="rrt")
                    nc.scalar.activation(rrt[:, :W0], lnt[:, :W0],
                                         AF.Exp, scale=-1.0)
                    # evac unnormalized PV
                    nc.vector.tensor_copy(o_sb[0:64, pair, :W0],
                                          pvs[0][0:64, :W0])
                    nc.vector.tensor_copy(o_sb[64:128, pair, :W0],
                                          pvs[1][0:64, :W0])
                    # broadcast 1/rowsum and normalize in place
                    rrb = pp_bc.tile([PP, 512], f32, tag="bc", name="rrb")
                    nc.tensor.matmul(rrb[0:64, :W0],
                                     lhsT=ones_row_bf[:, :64],
                                     rhs=rrt[0:1, :W0], start=True, stop=True)
                    nc.tensor.matmul(rrb[64:128, :W0],
                                     lhsT=ones_full[32:33, :64],
                                     rhs=rrt[32:33, :W0], start=True, stop=True)
                    rrs = rrsp.tile([PP, 512], bf16, tag="rrs", name="rrs")
                    nc.vector.tensor_copy(rrs[:, :W0], rrb[:, :W0])
                    nc.vector.tensor_tensor(
                        o_sb[:, pair, :W0], o_sb[:, pair, :W0],
                        rrs[:, :W0], OP.mult)

            # ---------------- persistent tiles ----------------
            xf_sb = bigp.tile([PP, 24, S], fp8, tag="big", name="xf_sb")
            nc.sync.dma_start(
                out=xf_sb, in_=dp["xfT"][:].rearrange("(ko p) m -> p ko m", p=PP))
            pos_sb = act.tile([PP, 6, S], bf16)
            nc.sync.dma_start(
                out=pos_sb, in_=dp["pos"][:].rearrange("(ko p) m -> p ko m", p=PP))
            h_sb = act.tile([PP, 6, S], f32)
            hb_sb = act.tile([PP, 6, S], bf16)
            h8_sb = act.tile([PP, 6, S], fp8)
            q_sb = act.tile([PP, 6, S], bf16)
            k_sb = act.tile([PP, 6, S], bf16)
            vT_sb = act.tile([PP, 4, 6 * 130], bf16)
            o_sb = act.tile([PP, 6, S], bf16)
            # ones columns in vT (positions static across layers)
            for pr in range(6):
                for sub in range(2):
                    nc.vector.memset(
                        vT_sb[:, :, pr * 130 + sub * 65 + 64:
                              pr * 130 + sub * 65 + 65], 1.0)

            # ---------------- patch embedding (fp8 DR) ----------------
            prewarm(AF.Ln)

            def embed_evac(m, ps):
                t = tmp.tile([PP, S], f32, tag="ev")
                nc.vector.tensor_scalar_mul(t, ps, INV_W8)
                nc.vector.tensor_tensor(h_sb[:, m, :], t, pos_sb[:, m, :],
                                        OP.add)
            dense("wk", 24, 6, xf_sb, S, embed_evac, use_fp8=True)

            if "dbg_h0" in dbg_ext:
                nc.sync.dma_start(out=dbg_ext["dbg_h0"][:], in_=h_sb)

            # ---------------- encoder layers ----------------
            for l in range(L):
                # LN1: stats + inv/64 row; mean-sub is folded into weights
                inv64_bf = ln_stats(h_sb, hb_sb, 6, S, want64=True)
                for k in range(6):
                    nc.vector.tensor_copy(h8_sb[:, k, :], hb_sb[:, k, :])

                qb = load_b(f"eqb{l}", 6)
                box = {}

                def mid_ln1(l=l, inv64_bf=inv64_bf, box=box):
                    def f():
                        box["inv64_sb"] = bcast_row(inv64_bf, S, f"i64_{l}")
                        box["ics"] = inv_cols(inv64_bf, 4, f"l{l}")
                    return f

                def qkv_evac(m, ps):
                    if m < 6:      # Q: *inv/64 then +bias
                        nc.vector.tensor_tensor(q_sb[:, m, :], ps,
                                                box["inv64_sb"], OP.mult)
                        nc.vector.tensor_scalar_add(q_sb[:, m, :], q_sb[:, m, :],
                                                    qb[:, m:m + 1])
                    else:          # K: *inv/64
                        nc.vector.tensor_tensor(k_sb[:, m - 6, :], ps,
                                                box["inv64_sb"], OP.mult)
                dense(f"eqk{l}", 6, 12, h8_sb, S, qkv_evac, use_fp8=True,
                      defer=6, mid=mid_ln1())

                # V (bf16 x64, token-major, *inv/64 at evac per-partition)
                wv = load_w(f"evw{l}", 6, D)
                v_dense(wv, hb_sb, vT_sb, 6, 4, D, box["ics"])

                attention(q_sb, k_sb, vT_sb, o_sb, NH, 4, allowed[l])

                pb = load_b(f"epb{l}", 6)

                def proj_evac(m, ps):
                    t = tmp.tile([PP, S], f32, tag="ev")
                    nc.vector.tensor_scalar_add(t, ps, pb[:, m:m + 1])
                    nc.gpsimd.tensor_tensor(h_sb[:, m, :], h_sb[:, m, :], t,
                                            OP.add)
                dense(f"eproj{l}", 6, 6, o_sb, S, proj_evac)

                # LN2 + MLP
                inv64_2bf = ln_stats(h_sb, hb_sb, 6, S, want64=True)
                for k in range(6):
                    nc.vector.tensor_copy(h8_sb[:, k, :], hb_sb[:, k, :])
                prewarm(AF.Gelu)

                g_sb = bigp.tile([PP, 24, S], bf16, tag="big", name="g_sb")
                m1b = load_b(f"em1b{l}", 24)
                box2 = {}

                def mid_ln2(l=l, inv64_2bf=inv64_2bf, box2=box2):
                    def f():
                        box2["inv"] = bcast_row(inv64_2bf, S, f"i642_{l}")
                    return f

                def gelu_evac(m, ps):
                    t = tmp.tile([PP, S], f32, tag="ev")
                    nc.vector.tensor_tensor(t, ps, box2["inv"], OP.mult)
                    nc.scalar.activation(g_sb[:, m, :], t, AF.Gelu,
                                         bias=m1b[:, m:m + 1])
                dense(f"em1{l}", 6, 24, h8_sb, S, gelu_evac, use_fp8=True,
                      defer=6, mid=mid_ln2())
                prewarm(AF.Ln)

                m2b = load_b(f"em2b{l}", 6)

                def mlp2_evac(m, ps):
                    t = tmp.tile([PP, S], f32, tag="ev")
                    nc.vector.tensor_scalar_add(t, ps, m2b[:, m:m + 1])
                    nc.gpsimd.tensor_tensor(h_sb[:, m, :], h_sb[:, m, :], t,
                                            OP.add)
                dense(f"em2{l}", 24, 6, g_sb, S, mlp2_evac)

                if f"dbg_he{l}" in dbg_ext:
                    nc.sync.dma_start(out=dbg_ext[f"dbg_he{l}"][:], in_=h_sb)

            # ---------------- encoder -> decoder ----------------
            for k in range(6):
                nc.vector.tensor_copy(hb_sb[:, k, :], h_sb[:, k, :])
            e2db = load_b("e2db", 4)
            memT_sb = act.tile([PP, 4, S], bf16)   # feature-major mem

            def e2d_evac(m, ps):
                nc.vector.tensor_scalar_add(memT_sb[:, m, :], ps,
                                            e2db[:, m:m + 1])
            dense("e2dw", 6, 4, hb_sb, S, e2d_evac)

            # ---------------- decoder ----------------
            qd_sb = act.tile([PP, 4, P], f32)      # decoder residual stream
            nc.sync.dma_start(
                out=qd_sb, in_=dp["dq"][:].rearrange("(ko p) m -> p ko m", p=PP))

            qdb_sb = act.tile([PP, 4, P], bf16)
            Qd_sb = act.tile([PP, 4, P], bf16)
            Kd_sb = act.tile([PP, 4, S], bf16)
            vTd_sb = act.tile([PP, 4, 4 * 130], bf16)
            od_sb = act.tile([PP, 4, P], bf16)
            gd_sb = act.tile([PP, 16, P], bf16)
            for pr in range(4):
                for sub in range(2):
                    nc.vector.memset(
                        vTd_sb[:, :, pr * 130 + sub * 65 + 64:
                               pr * 130 + sub * 65 + 65], 1.0)

            for l in range(LD):

                def kd_evac(m, ps):
                    nc.vector.tensor_copy(Kd_sb[:, m, :], ps)
                dense(f"dwk{l}", 4, 4, memT_sb, S, kd_evac)

                wvd = load_w(f"dwv{l}", 4, DD)
                v_dense(wvd, memT_sb, vTd_sb, 4, 4, DD, None)

                inv_dbf = ln_stats(qd_sb, qdb_sb, 4, P)
                qbd = load_b(f"dqb{l}", 4)
                boxd = {}

                def mid_lnd(l=l, inv_dbf=inv_dbf, boxd=boxd):
                    def f():
                        boxd["inv"] = bcast_row(inv_dbf, P, f"id_{l}")
                    return f

                def qd_evac(m, ps):
                    nc.vector.tensor_tensor(Qd_sb[:, m, :], ps,
                                            boxd["inv"][:, :P], OP.mult)
                    nc.vector.tensor_scalar_add(Qd_sb[:, m, :], Qd_sb[:, m, :],
                                                qbd[:, m:m + 1])
                dense(f"dwq{l}", 4, 4, qdb_sb, P, qd_evac,
                      defer=4, mid=mid_lnd())

                attention(Qd_sb, Kd_sb, vTd_sb, od_sb, NHD, 1,
                          [(0, 1, 2, 3)])

                obd = load_b(f"dob{l}", 4)

                def od_evac(m, ps):
                    t = tmp.tile([PP, S], f32, tag="ev")
                    nc.vector.tensor_scalar_add(t[:, :P], ps, obd[:, m:m + 1])
                    nc.gpsimd.tensor_tensor(qd_sb[:, m, :], qd_sb[:, m, :],
                                            t[:, :P], OP.add)
                dense(f"dwo{l}", 4, 4, od_sb, P, od_evac)

                inv_d2bf = ln_stats(qd_sb, qdb_sb, 4, P)
                prewarm(AF.Gelu)
                m1bd = load_b(f"dm1b{l}", 16)
                boxd2 = {}

                def mid_lnd2(l=l, inv_d2bf=inv_d2bf, boxd2=boxd2):
                    def f():
                        boxd2["inv"] = bcast_row(inv_d2bf, P, f"id2_{l}")
                    return f

                def gelud_evac(m, ps):
                    t = tmp.tile([PP, S], f32, tag="ev")
                    nc.vector.tensor_tensor(t[:, :P], ps, boxd2["inv"][:, :P],
                                            OP.mult)
                    nc.scalar.activation(gd_sb[:, m, :], t[:, :P], AF.Gelu,
                                         bias=m1bd[:, m:m + 1])
                dense(f"dm1{l}", 4, 16, qdb_sb, P, gelud_evac,
                      defer=6, mid=mid_lnd2())
                prewarm(AF.Ln)

                m2bd = load_b(f"dm2b{l}", 4)

                def mlp2d_evac(m, ps):
                    t = tmp.tile([PP, S], f32, tag="ev")
                    nc.vector.tensor_scalar_add(t[:, :P], ps, m2bd[:, m:m + 1])
                    nc.gpsimd.tensor_tensor(qd_sb[:, m, :], qd_sb[:, m, :],
                                            t[:, :P], OP.add)
                dense(f"dm2{l}", 16, 4, gd_sb, P, mlp2d_evac)

            # ---------------- head ----------------
            inv_hbf = ln_stats(qd_sb, qdb_sb, 4, P)
            ics_h = inv_cols(inv_hbf, 1, "hd")
            wh = load_w("hw", 4, OUT)
            hbb_sb = act.tile([PP, OUT], f32)
            nc.sync.dma_start(out=hbb_sb, in_=dp["hbb"][:])
            out_sb = act.tile([P, OUT], f32)
            for nchunk in range(2):
                ncs = slice(nchunk * 384, (nchunk + 1) * 384)
                ps = pp_mm.tile([PP, 512], f32, tag="mm")
                for k in range(4):
                    nc.tensor.matmul(ps[:, :384],
                                     lhsT=qdb_sb[:, k, :],
                                     rhs=wh[:, k, ncs],
                                     start=(k == 0), stop=(k == 3))
                t = tmp.tile([PP, S], f32, tag="ev")
                nc.vector.tensor_scalar_mul(t[:, :384], ps[:, :384],
                                            ics_h[:, 0:1])
                nc.vector.tensor_tensor(out_sb[:, ncs], t[:, :384],
                                        hbb_sb[:, ncs], OP.add)
            nc.sync.dma_start(out=out_ext[:], in_=out_sb)

    return nc


# ------------------------------------------------------------------
# entry point
# ------------------------------------------------------------------

def kernel(dbg=(), _trace=False, _tmpdir=None, _full=False, **inputs):
    from concourse.bass_utils import run_bass_kernel_spmd

    w, xfT_f8, allowed = _prep(inputs)
    nc = _build(allowed, dbg=dbg)
    in_maps = []
    for b in range(B):
        m = dict(w)
        m["xfT"] = xfT_f8[b]
        in_maps.append(m)
    res = run_bass_kernel_spmd(nc, in_maps, core_ids=list(range(8)),
                               trace=_trace, tmpdir=_tmpdir)
    out = np.stack([np.asarray(res.results[i]["out"]) for i in range(B)])
    if dbg or _full:
        dbgs = {name: np.stack([np.asarray(res.results[i][name])
                                for i in range(B)]) for name in dbg}
        return out.astype(np.float32), dbgs, res
    return out.astype(np.float32)
